# revision 1
# baseline (speedup 1.0000x reference)
"""Trainium2 Bass kernel for a 2-layer GAT + global max pool + linear head.

Contract: kernel(**inputs) takes FULL unsharded inputs (as produced by
reference.setup_inputs) and returns the FULL [N_GRAPHS, N_CLASSES] float32
output. Internally: shards nodes (and their incident edges, 1D partitioned
by destination) across 8 NeuronCores, replicates the small GAT weights,
AllGathers the per-layer node-feature tables, and AllReduces the pooled
per-graph maxima.

Self-contained: hardcodes all shapes; reads nothing from /root/problem.
"""
import sys

sys.path.insert(0, "/opt/trn_rl_repo")
sys.path.insert(0, "/opt/pypackages")

import numpy as np

# ---------------------------------------------------------------- constants
FULL_CFG = dict(
    N=50000, IN=128, HID=32, OUT=64, HEADS=4, NCLS=32, NG=64,
    NC=8, NEG=0.2,
)

P = 128
BATCHED_GATHER = False


def _derive(cfg):
    cfg = dict(cfg)
    assert cfg["N"] % cfg["NC"] == 0
    cfg["NLOC"] = cfg["N"] // cfg["NC"]
    cfg["TILES"] = -(-cfg["NLOC"] // P)
    cfg["TPAD"] = cfg["TILES"] * P
    cfg["VPAD"] = cfg["NC"] * cfg["TPAD"]
    cfg["CL1"] = cfg["HEADS"] * cfg["HID"]   # 128
    cfg["CL2"] = cfg["HEADS"] * cfg["OUT"]   # 256
    cfg["R1"] = cfg["CL1"] + 8               # table1 row width (fp16)
    cfg["R2"] = cfg["CL2"] + 8               # table2 row width (fp16)
    assert cfg["CL1"] == P and cfg["IN"] == P
    return cfg


# ---------------------------------------------------------------- host prep
def _preprocess(cfg, x, edge_index, batch):
    """Integer/layout-only host preprocessing. Returns (meta, per_core, shared)."""
    NC, N, NLOC, TILES, TPAD = cfg["NC"], cfg["N"], cfg["NLOC"], cfg["TILES"], cfg["TPAD"]
    src = np.concatenate([edge_index[0].astype(np.int64), np.arange(N, dtype=np.int64)])
    dst = np.concatenate([edge_index[1].astype(np.int64), np.arange(N, dtype=np.int64)])
    order = np.argsort(dst, kind="stable")
    srcs, dsts = src[order], dst[order]

    remap_src = (srcs // NLOC) * TPAD + (srcs % NLOC)
    remap_dst = (dsts // NLOC) * TPAD + (dsts % NLOC)

    core_bounds = np.searchsorted(dsts, np.arange(NC + 1) * NLOC)
    # per-(core,tile) edge counts
    counts = np.zeros((NC, TILES), np.int64)
    tile_of = ((dsts % NLOC) // P).astype(np.int64)
    for k in range(NC):
        sl = slice(core_bounds[k], core_bounds[k + 1])
        counts[k] = np.bincount(tile_of[sl], minlength=TILES)
    W = np.maximum(1, -(-counts.max(axis=0) // P)).astype(np.int64)  # chunks per tile
    bases = np.zeros(TILES + 1, np.int64)
    bases[1:] = np.cumsum(P * 3 * W)
    ETOT = int(bases[-1])

    eidx = np.zeros((NC, ETOT), np.int32)
    for k in range(NC):
        sl = slice(core_bounds[k], core_bounds[k + 1])
        rs, rd = remap_src[sl], remap_dst[sl]
        loc = (dsts[sl] % NLOC).astype(np.int64)
        tl = tile_of[sl]
        tile_starts = np.searchsorted(tl, np.arange(TILES + 1))
        for t in range(TILES):
            Wt = int(W[t])
            a, b = tile_starts[t], tile_starts[t + 1]
            n = b - a
            blk_src = np.full((P, 3 * Wt), 0, np.int32)
            blk_src[:, 0:Wt] = ((k + 1) % NC) * TPAD        # pad src -> valid zero row
            blk_src[:, Wt:2 * Wt] = ((k + 1) % NC) * TPAD    # pad dst-gather idx
            blk_src[:, 2 * Wt:3 * Wt] = TPAD                 # pad dloc source
            if n:
                jj, pp = np.divmod(np.arange(n), P)
                blk_src[pp, jj] = rs[a:b].astype(np.int32)
                blk_src[pp, Wt + jj] = rd[a:b].astype(np.int32)
                blk_src[pp, 2 * Wt + jj] = loc[a:b].astype(np.int32)
            eidx[k, bases[t]:bases[t + 1]] = blk_src.reshape(-1)

    # pooling segments per (core, tile): runs of equal batch id
    POOLP, NG = cfg["OUT"], cfg["NG"]
    runs = [[[] for _ in range(TILES)] for _ in range(NC)]
    SEG = 1
    for k in range(NC):
        bl = batch[k * NLOC:(k + 1) * NLOC].astype(np.int64)
        for t in range(TILES):
            lo = t * P
            hi = min((t + 1) * P, NLOC)
            seg = bl[lo:hi]
            if len(seg) == 0:
                continue
            cuts = np.flatnonzero(np.diff(seg)) + 1
            edges = np.concatenate([[0], cuts, [len(seg)]])
            for i in range(len(edges) - 1):
                runs[k][t].append((int(seg[edges[i]]), int(edges[i]), int(edges[i + 1])))
            SEG = max(SEG, len(edges) - 1)
    STOT = TILES * SEG
    segmask = np.zeros((NC, TILES, POOLP, SEG * P), np.float16)
    M = np.zeros((NC, POOLP, NG * STOT), np.float16)
    for k in range(NC):
        for t in range(TILES):
            for s, (g, lo, hi) in enumerate(runs[k][t]):
                segmask[k, t, :, s * P + lo:s * P + hi] = 1.0
                M[k, :, g * STOT + t * SEG + s] = 1.0

    # x transposed/padded/cast per core
    xT16 = np.zeros((NC, P, TPAD), np.float16)
    for k in range(NC):
        xl = x[k * NLOC:(k + 1) * NLOC].astype(np.float16)
        xT16[k, :, :NLOC] = xl.T

    meta = dict(W=[int(w) for w in W], bases=[int(b) for b in bases], ETOT=ETOT,
                SEG=SEG, STOT=STOT)
    per_core = [dict(xT16=xT16[k], eidx=eidx[k],
                     segmask=segmask[k].reshape(TILES * POOLP, SEG * P),
                     Mfull=M[k]) for k in range(NC)]
    return meta, per_core


def _weights_host(cfg, W1, a_s1, a_d1, b1, W2, a_s2, a_d2, b2, Wl, bl):
    HEADS, HID, OUT = cfg["HEADS"], cfg["HID"], cfg["OUT"]
    CL1, CL2, NCLS, NG = cfg["CL1"], cfg["CL2"], cfg["NCLS"], cfg["NG"]
    f16, f32 = np.float16, np.float32
    A1 = np.zeros((CL1, 8), f16)
    for h in range(HEADS):
        A1[h * HID:(h + 1) * HID, h] = a_s1[h].astype(f16)
        A1[h * HID:(h + 1) * HID, 4 + h] = a_d1[h].astype(f16)
    A2 = np.zeros((CL2, 8), f16)
    for h in range(HEADS):
        A2[h * OUT:(h + 1) * OUT, h] = a_s2[h].astype(f16)
        A2[h * OUT:(h + 1) * OUT, 4 + h] = a_d2[h].astype(f16)
    sh = dict(
        IOTA16=np.tile(np.arange(P, dtype=f16)[None, :], (P, 1)),
        W1T16=W1.T.astype(f16).copy(),                 # [IN, CL1]
        W116=W1.astype(f16).copy(),                    # [CL1, IN]
        A1blk=A1,                                      # [CL1, 8]
        W2T16=W2.T.astype(f16).copy(),                 # [CL1, CL2]
        W216a=W2[:P].astype(f16).copy(),               # [128, CL1]
        W216b=W2[P:].astype(f16).copy(),
        A2blka=A2[:P].copy(), A2blkb=A2[P:].copy(),
        b1row=np.tile(b1.astype(f32)[None, :], (P, 1)),          # [128, CL1]
        b2row=np.tile(b2.astype(f32)[None, :], (P, 1)),          # [128, OUT]
        blrow=np.tile(bl.astype(f32)[None, :], (NG, 1)),         # [NG, NCLS]
        WlT32=Wl.T.astype(f32).copy(),                 # [OUT, NCLS]
    )
    return sh


# ---------------------------------------------------------------- program
def _build(cfg, meta):
    import concourse.bass as bass
    import concourse.bacc as bacc
    import concourse.tile as tile
    from concourse import mybir
    from concourse.masks import make_identity

    f16, f32, i32 = mybir.dt.float16, mybir.dt.float32, mybir.dt.int32
    Alu = mybir.AluOpType
    Act = mybir.ActivationFunctionType
    NC, TILES, TPAD, VPAD = cfg["NC"], cfg["TILES"], cfg["TPAD"], cfg["VPAD"]
    CL1, CL2, R1, R2 = cfg["CL1"], cfg["CL2"], cfg["R1"], cfg["R2"]
    OUT, NG, NCLS, NEG = cfg["OUT"], cfg["NG"], cfg["NCLS"], cfg["NEG"]
    HEADS, HID = cfg["HEADS"], cfg["HID"]
    SEG, STOT = meta["SEG"], meta["STOT"]
    Wl_, bases = meta["W"], meta["bases"]
    POOLP = OUT
    OFF = 1024.0  # masked-max offset

    nc = bacc.Bacc("TRN2", target_bir_lowering=False, debug=False, num_devices=NC)

    pr = {}
    def param(name, shape, dt, out=False):
        pr[name] = nc.declare_dram_parameter(name, list(shape), dt, isOutput=out)
        return pr[name]

    param("xT16", [P, TPAD], f16)
    param("eidx", [meta["ETOT"]], i32)
    param("segmask", [TILES * POOLP, SEG * P], f16)
    param("Mfull", [POOLP, NG * STOT], f16)
    param("IOTA16", [P, P], f16)
    param("W1T16", [P, CL1], f16)
    param("W116", [CL1, P], f16)
    param("A1blk", [CL1, 8], f16)
    param("W2T16", [CL1, CL2], f16)
    param("W216a", [P, CL1], f16)
    param("W216b", [CL2 - P, CL1], f16)
    param("A2blka", [P, 8], f16)
    param("A2blkb", [CL2 - P, 8], f16)
    param("b1row", [P, CL1], f32)
    param("b2row", [P, OUT], f32)
    param("blrow", [NG, NCLS], f32)
    param("WlT32", [OUT, NCLS], f32)
    out_ext = param("out_logits", [NG, NCLS], f32, out=True)

    rg = [list(range(NC))]

    with tile.TileContext(nc) as tc:
        with tc.tile_pool(name="const", bufs=1) as cp, \
             tc.tile_pool(name="dram", bufs=1, space="DRAM") as dp:

            # ---- constants to SBUF
            def ld(name, shape, dt):
                t_ = cp.tile(list(shape), dt, name="c_" + name)
                nc.sync.dma_start(t_[:], pr[name][:])
                return t_
            xT = ld("xT16", [P, TPAD], f16)
            iota = ld("IOTA16", [P, P], f16)
            W1T = ld("W1T16", [P, CL1], f16)
            W116 = ld("W116", [CL1, P], f16)
            A1blk = ld("A1blk", [CL1, 8], f16)
            W2T = ld("W2T16", [CL1, CL2], f16)
            W216a = ld("W216a", [P, CL1], f16)
            W216b = ld("W216b", [CL2 - P, CL1], f16)
            A2blka = ld("A2blka", [P, 8], f16)
            A2blkb = ld("A2blkb", [CL2 - P, 8], f16)
            b1row = ld("b1row", [P, CL1], f32)
            b2row = ld("b2row", [P, OUT], f32)
            blrow = ld("blrow", [NG, NCLS], f32)
            WlT32 = ld("WlT32", [OUT, NCLS], f32)
            Mfull = ld("Mfull", [POOLP, NG * STOT], f16)

            ident = cp.tile([P, P], f32)
            make_identity(nc, ident[:])
            zOUT = cp.tile([P, OUT], f32)
            nc.vector.memset(zOUT[:], 0.0)

            # ---- B1/B2 fold matrices (al = x @ B per layer input)
            B1 = cp.tile([P, 8], f16)
            B2 = cp.tile([P, 8], f16)
            with tc.tile_pool(name="cps", bufs=1, space="PSUM") as cps:
                B1ps = cps.tile([P, 8], f32)
                nc.tensor.matmul(B1ps[:], lhsT=W116[:], rhs=A1blk[:], start=True, stop=True)
                nc.vector.tensor_copy(out=B1[:], in_=B1ps[:])
                B2ps = cps.tile([P, 8], f32)
                nc.tensor.matmul(B2ps[:], lhsT=W216a[:], rhs=A2blka[:], start=True, stop=False)
                nc.tensor.matmul(B2ps[:], lhsT=W216b[:], rhs=A2blkb[:], start=False, stop=True)
                nc.vector.tensor_copy(out=B2[:], in_=B2ps[:])

            # ---- DRAM internals
            tb1_loc = dp.tile([TPAD, R1], f16)
            tb1_full = dp.tile([VPAD, R1], f16)
            ad1_loc = dp.tile([TPAD, 4], f16)
            ad1_full = dp.tile([VPAD, 4], f16)
            tb2_loc = dp.tile([TPAD, R2], f16)
            tb2_full = dp.tile([VPAD, R2], f16)
            ad2_loc = dp.tile([TPAD, 4], f16)
            ad2_full = dp.tile([VPAD, 4], f16)
            ar_in = dp.tile([POOLP, NG], f32)
            ar_out = dp.tile([POOLP, NG], f32)

            # ================= phase B: table1 build =================
            with tc.tile_pool(name="phB", bufs=3) as pb, \
                 tc.tile_pool(name="phBps", bufs=2, space="PSUM") as pbps:
                for t in range(TILES):
                    xsl = xT[:, t * P:(t + 1) * P]
                    h1ps = pbps.tile([P, CL1], f32, tag="h1ps")
                    nc.tensor.matmul(h1ps[:], lhsT=xsl, rhs=W1T[:], start=True, stop=True)
                    alps = pbps.tile([P, 8], f32, tag="alps")
                    nc.tensor.matmul(alps[:], lhsT=xsl, rhs=B1[:], start=True, stop=True)
                    h116 = pb.tile([P, CL1], f16, tag="h116")
                    nc.vector.tensor_copy(out=h116[:], in_=h1ps[:])
                    al16 = pb.tile([P, 8], f16, tag="al16")
                    nc.vector.tensor_copy(out=al16[:], in_=alps[:])
                    nc.sync.dma_start(tb1_loc[t * P:(t + 1) * P, 0:CL1], h116[:])
                    nc.sync.dma_start(tb1_loc[t * P:(t + 1) * P, CL1:CL1 + 8], al16[:, 0:8])
                    nc.sync.dma_start(ad1_loc[t * P:(t + 1) * P, 0:4], al16[:, 4:8])

            nc.gpsimd.collective_compute(
                "AllGather", mybir.AluOpType.bypass, replica_groups=rg,
                ins=[tb1_loc[:].opt()], outs=[tb1_full[:].opt()])
            nc.gpsimd.collective_compute(
                "AllGather", mybir.AluOpType.bypass, replica_groups=rg,
                ins=[ad1_loc[:].opt()], outs=[ad1_full[:].opt()])

            # ================= edge phases =================
            def edge_tile(pe, pp, t, CL, ROW, table, adtab, epilogue):
                RW = CL + 4
                Wt = Wl_[t]
                base = bases[t]
                idx = pe.tile([P, 3 * Wt], i32, tag="idx")
                nc.sync.dma_start(
                    idx[:], pr["eidx"][base:base + P * 3 * Wt].rearrange("(p w) -> p w", p=P))
                dlf = pe.tile([P, Wt], f32, tag="dlf")
                nc.vector.tensor_copy(out=dlf[:], in_=idx[:, 2 * Wt:3 * Wt])
                dloc = pe.tile([P, Wt], f16, tag="dloc")
                nc.vector.tensor_scalar(out=dloc[:], in0=dlf[:], scalar1=float(t * P),
                                        scalar2=None, op0=Alu.subtract)
                hg = pe.tile([P, Wt * ROW], f16, tag="hg")
                if BATCHED_GATHER:
                    nc.gpsimd.indirect_dma_start(
                        out=hg[:].rearrange("p (w r) -> p w r", w=Wt), out_offset=None,
                        in_=table[:], in_offset=bass.IndirectOffsetOnAxis(ap=idx[:, 0:Wt], axis=0))
                else:
                    for j in range(Wt):
                        nc.gpsimd.indirect_dma_start(
                            out=hg[:, j * ROW:(j + 1) * ROW], out_offset=None,
                            in_=table[:], in_offset=bass.IndirectOffsetOnAxis(ap=idx[:, j:j + 1], axis=0))
                adg = pe.tile([P, Wt * 4], f16, tag="adg")
                if BATCHED_GATHER:
                    nc.gpsimd.indirect_dma_start(
                        out=adg[:].rearrange("p (w r) -> p w r", w=Wt), out_offset=None,
                        in_=adtab[:], in_offset=bass.IndirectOffsetOnAxis(ap=idx[:, Wt:2 * Wt], axis=0))
                else:
                    for j in range(Wt):
                        nc.gpsimd.indirect_dma_start(
                            out=adg[:, j * 4:(j + 1) * 4], out_offset=None,
                            in_=adtab[:], in_offset=bass.IndirectOffsetOnAxis(ap=idx[:, Wt + j:Wt + j + 1], axis=0))
                mask = pe.tile([P, Wt * P], f16, tag="mask")
                nc.vector.tensor_tensor(
                    out=mask[:].rearrange("p (w d) -> p w d", w=Wt),
                    in0=dloc[:, :, None].to_broadcast([P, Wt, P]),
                    in1=iota[:][:, None, :].to_broadcast([P, Wt, P]),
                    op=Alu.is_equal)
                hg3 = hg[:].rearrange("p (w r) -> p w r", w=Wt)
                adg3 = adg[:].rearrange("p (w r) -> p w r", w=Wt)
                sc = pe.tile([P, Wt * 4], f32, tag="sc")
                nc.vector.tensor_tensor(out=sc[:].rearrange("p (w h) -> p w h", w=Wt),
                                        in0=hg3[:, :, CL:CL + 4], in1=adg3[:, :, 0:4],
                                        op=Alu.add)
                lr = pe.tile([P, Wt * 4], f32, tag="lr")
                nc.vector.scalar_tensor_tensor(out=lr[:], in0=sc[:], scalar=NEG,
                                               in1=sc[:], op0=Alu.mult, op1=Alu.max)
                ex = pe.tile([P, Wt * 4], f16, tag="ex")
                nc.scalar.activation(ex[:], lr[:], Act.Exp)
                rhs = pe.tile([P, Wt * RW], f16, tag="rhs")
                rhs3 = rhs[:].rearrange("p (w r) -> p w r", w=Wt)
                nc.vector.tensor_tensor(
                    out=rhs3[:, :, 0:CL].rearrange("p w (h c) -> p w h c", h=HEADS),
                    in0=hg3[:, :, 0:CL].rearrange("p w (h c) -> p w h c", h=HEADS),
                    in1=ex[:].rearrange("p (w h) -> p w h", w=Wt)[:, :, :, None]
                        .to_broadcast([P, Wt, HEADS, CL // HEADS]),
                    op=Alu.mult)
                nc.vector.tensor_copy(out=rhs3[:, :, CL:CL + 4],
                                      in_=ex[:].rearrange("p (w h) -> p w h", w=Wt))
                num = pp.tile([P, RW], f32, tag="num")
                for j in range(Wt):
                    nc.tensor.matmul(num[:], lhsT=mask[:, j * P:(j + 1) * P],
                                     rhs=rhs[:, j * RW:(j + 1) * RW],
                                     start=(j == 0), stop=(j == Wt - 1))
                epilogue(t, num)

            # ---- layer 1 (+ table2 build in epilogue)
            with tc.tile_pool(name="ph1", bufs=3) as p1, \
                 tc.tile_pool(name="ph1b", bufs=2) as p1b, \
                 tc.tile_pool(name="ph1num", bufs=2, space="PSUM") as p1num, \
                 tc.tile_pool(name="ph1ps", bufs=1, space="PSUM") as p1ps:

                def epi1(t, num):
                    den = p1b.tile([P, 4], f32, tag="den")
                    nc.vector.tensor_scalar_add(out=den[:], in0=num[:, CL1:CL1 + 4], scalar1=1e-16)
                    rden = p1b.tile([P, 4], f32, tag="rden")
                    nc.vector.reciprocal(rden[:], den[:])
                    u32 = p1b.tile([P, CL1], f32, tag="u32")
                    for h in range(HEADS):
                        nc.vector.scalar_tensor_tensor(
                            out=u32[:, h * HID:(h + 1) * HID],
                            in0=num[:, h * HID:(h + 1) * HID],
                            scalar=rden[:, h:h + 1],
                            in1=b1row[:, h * HID:(h + 1) * HID],
                            op0=Alu.mult, op1=Alu.add)
                    ur = p1b.tile([P, CL1], f32, tag="ur")
                    nc.scalar.activation(ur[:], u32[:], Act.Relu)
                    uTps = p1ps.tile([P, P], f32, tag="uTps")
                    nc.tensor.transpose(out=uTps[:], in_=ur[:], identity=ident[:])
                    uT16 = p1b.tile([P, P], f16, tag="uT16")
                    nc.vector.tensor_copy(out=uT16[:], in_=uTps[:])
                    h2ps = p1ps.tile([P, CL2], f32, tag="h2ps")
                    nc.tensor.matmul(h2ps[:], lhsT=uT16[:], rhs=W2T[:], start=True, stop=True)
                    al2ps = p1ps.tile([P, 8], f32, tag="al2ps")
                    nc.tensor.matmul(al2ps[:], lhsT=uT16[:], rhs=B2[:], start=True, stop=True)
                    h216 = p1b.tile([P, CL2], f16, tag="h216")
                    nc.vector.tensor_copy(out=h216[:], in_=h2ps[:])
                    al216 = p1b.tile([P, 8], f16, tag="al216")
                    nc.vector.tensor_copy(out=al216[:], in_=al2ps[:])
                    nc.sync.dma_start(tb2_loc[t * P:(t + 1) * P, 0:CL2], h216[:])
                    nc.sync.dma_start(tb2_loc[t * P:(t + 1) * P, CL2:CL2 + 8], al216[:, 0:8])
                    nc.sync.dma_start(ad2_loc[t * P:(t + 1) * P, 0:4], al216[:, 4:8])

                for t in range(TILES):
                    edge_tile(p1, p1num, t, CL1, R1, tb1_full, ad1_full, epi1)

            nc.gpsimd.collective_compute(
                "AllGather", mybir.AluOpType.bypass, replica_groups=rg,
                ins=[tb2_loc[:].opt()], outs=[tb2_full[:].opt()])
            nc.gpsimd.collective_compute(
                "AllGather", mybir.AluOpType.bypass, replica_groups=rg,
                ins=[ad2_loc[:].opt()], outs=[ad2_full[:].opt()])

            # ---- layer 2 + pooling
            partial = cp.tile([POOLP, STOT], f32)
            with tc.tile_pool(name="ph2", bufs=3) as p2, \
                 tc.tile_pool(name="ph2b", bufs=2) as p2b, \
                 tc.tile_pool(name="ph2num", bufs=2, space="PSUM") as p2num, \
                 tc.tile_pool(name="ph2ps", bufs=1, space="PSUM") as p2ps:

                def epi2(t, num):
                    den = p2b.tile([P, 4], f32, tag="den2")
                    nc.vector.tensor_scalar_add(out=den[:], in0=num[:, CL2:CL2 + 4], scalar1=1e-16)
                    rden = p2b.tile([P, 4], f32, tag="rden2")
                    nc.vector.reciprocal(rden[:], den[:])
                    acc0 = p2b.tile([P, OUT], f32, tag="acc0")
                    nc.vector.scalar_tensor_tensor(
                        out=acc0[:], in0=num[:, 0:OUT], scalar=rden[:, 0:1],
                        in1=zOUT[:], op0=Alu.mult, op1=Alu.add)
                    acc1 = p2b.tile([P, OUT], f32, tag="acc1")
                    nc.vector.scalar_tensor_tensor(
                        out=acc1[:], in0=num[:, OUT:2 * OUT], scalar=rden[:, 1:2],
                        in1=acc0[:], op0=Alu.mult, op1=Alu.add)
                    acc2 = p2b.tile([P, OUT], f32, tag="acc2")
                    nc.vector.scalar_tensor_tensor(
                        out=acc2[:], in0=num[:, 2 * OUT:3 * OUT], scalar=rden[:, 2:3],
                        in1=acc1[:], op0=Alu.mult, op1=Alu.add)
                    acc3 = p2b.tile([P, OUT], f32, tag="acc3")
                    nc.vector.scalar_tensor_tensor(
                        out=acc3[:], in0=num[:, 3 * OUT:4 * OUT], scalar=rden[:, 3:4],
                        in1=acc2[:], op0=Alu.mult, op1=Alu.add)
                    o2 = p2b.tile([P, OUT], f32, tag="o2")
                    nc.vector.scalar_tensor_tensor(
                        out=o2[:], in0=acc3[:], scalar=1.0 / HEADS,
                        in1=b2row[:], op0=Alu.mult, op1=Alu.add)
                    o2Tps = p2ps.tile([OUT, P], f32, tag="o2Tps")
                    nc.tensor.transpose(out=o2Tps[:], in_=o2[:], identity=ident[:])
                    o2T = p2b.tile([OUT, P], f32, tag="o2T")
                    nc.vector.tensor_copy(out=o2T[:], in_=o2Tps[:])
                    sm = p2b.tile([POOLP, SEG * P], f16, tag="sm")
                    nc.sync.dma_start(sm[:], pr["segmask"][t * POOLP:(t + 1) * POOLP, :])
                    for s in range(SEG):
                        tmp = p2b.tile([POOLP, P], f32, tag="ptmp")
                        nc.vector.scalar_tensor_tensor(
                            out=tmp[:], in0=o2T[:], scalar=OFF,
                            in1=sm[:, s * P:(s + 1) * P], op0=Alu.add, op1=Alu.mult)
                        nc.vector.reduce_max(partial[:, t * SEG + s:t * SEG + s + 1],
                                             tmp[:], axis=mybir.AxisListType.X)

                for t in range(TILES):
                    edge_tile(p2, p2num, t, CL2, R2, tb2_full, ad2_full, epi2)

            # ---- combine partials -> per-graph max -> AllReduce -> head
            with tc.tile_pool(name="fin", bufs=1) as fp, \
                 tc.tile_pool(name="finps", bufs=1, space="PSUM") as fps:
                tmpg = fp.tile([POOLP, NG * STOT], f32)
                nc.vector.scalar_tensor_tensor(
                    out=tmpg[:], in0=partial[:][:, None, :].to_broadcast([POOLP, NG, STOT]),
                    scalar=0.0, in1=Mfull[:].rearrange("p (g s) -> p g s", g=NG),
                    op0=Alu.add, op1=Alu.mult)
                pooled = fp.tile([POOLP, NG], f32)
                nc.vector.reduce_max(pooled[:],
                                     tmpg[:].rearrange("p (g s) -> p g s", g=NG),
                                     axis=mybir.AxisListType.X)
                pooled2 = fp.tile([POOLP, NG], f32)
                nc.vector.tensor_scalar_add(out=pooled2[:], in0=pooled[:], scalar1=-OFF)
                nc.sync.dma_start(ar_in[:], pooled2[:])
                nc.gpsimd.collective_compute(
                    "AllReduce", mybir.AluOpType.max, replica_groups=rg,
                    ins=[ar_in[:].opt()], outs=[ar_out[:].opt()])
                pooledF = fp.tile([POOLP, NG], f32)
                nc.sync.dma_start(pooledF[:], ar_out[:])
                lps = fps.tile([NG, NCLS], f32)
                nc.tensor.matmul(lps[:], lhsT=pooledF[:], rhs=WlT32[:], start=True, stop=True)
                logits = fp.tile([NG, NCLS], f32)
                nc.vector.tensor_tensor(out=logits[:], in0=lps[:], in1=blrow[:], op=Alu.add)
                nc.sync.dma_start(out_ext[:], logits[:])

    nc.compile()
    return nc


# ---------------------------------------------------------------- runner
def _make_in_maps(cfg, per_core, shared):
    maps = []
    for k in range(cfg["NC"]):
        m = dict(shared)
        m.update(per_core[k])
        maps.append({k_: np.ascontiguousarray(v) for k_, v in m.items()})
    return maps


def _kernel_impl(inputs, trace=False, trace_kwargs=None):
    cfg = _derive(FULL_CFG)
    x = np.asarray(inputs["x"], np.float32)
    edge_index = np.asarray(inputs["edge_index"]).astype(np.int64)
    batch = np.asarray(inputs["batch"]).astype(np.int64)
    meta, per_core = _preprocess(cfg, x, edge_index, batch)
    shared = _weights_host(
        cfg,
        np.asarray(inputs["W1"], np.float32), np.asarray(inputs["a_src1"], np.float32),
        np.asarray(inputs["a_dst1"], np.float32), np.asarray(inputs["b1"], np.float32),
        np.asarray(inputs["W2"], np.float32), np.asarray(inputs["a_src2"], np.float32),
        np.asarray(inputs["a_dst2"], np.float32), np.asarray(inputs["b2"], np.float32),
        np.asarray(inputs["Wl"], np.float32), np.asarray(inputs["bl"], np.float32))
    nc = _build(cfg, meta)
    in_maps = _make_in_maps(cfg, per_core, shared)
    from concourse.bass_utils import run_bass_kernel_spmd
    res = run_bass_kernel_spmd(nc, in_maps, core_ids=list(range(cfg["NC"])),
                               trace=trace, **(trace_kwargs or {}))
    return np.asarray(res.results[0]["out_logits"], np.float32), res


def kernel(**inputs):
    return _kernel_impl(inputs)[0]



# revision 26
# speedup vs baseline: 1.3607x; 1.3607x over previous
"""Trainium2 Bass kernel for a 2-layer GAT + global max pool + linear head.

Contract: kernel(**inputs) takes FULL unsharded inputs (as produced by
reference.setup_inputs) and returns the FULL [N_GRAPHS, N_CLASSES] float32
output. Internally: shards nodes (and their incident edges, 1D partitioned
by destination) across 8 NeuronCores, replicates the small GAT weights,
AllGathers the per-layer node-feature tables (chunked, overlapped with
compute), and AllReduces the pooled per-graph maxima.

Edge gathers use bulk dma_gather (SWDGE) ops: per destination tile, two
source-row gathers (table split at 32768 rows for int16 indices) plus one
destination sub-row gather for the attention-dst coefficients.

Self-contained: hardcodes all shapes; reads nothing from /root/problem.
"""
import sys

sys.path.insert(0, "/opt/trn_rl_repo")
sys.path.insert(0, "/opt/pypackages")

import numpy as np

# ---------------------------------------------------------------- constants
FULL_CFG = dict(
    N=50000, IN=128, HID=32, OUT=64, HEADS=4, NCLS=32, NG=64,
    NC=8, NEG=0.2,
)

P = 128
HALF = 32768          # int16 index limit for dma_gather
NCHUNK = 4            # AllGather chunks per table
STUB_GATHER = False   # debug: replace gathers with memsets


def _derive(cfg):
    cfg = dict(cfg)
    assert cfg["N"] % cfg["NC"] == 0
    cfg["NLOC"] = cfg["N"] // cfg["NC"]
    cfg["TILES"] = -(-cfg["NLOC"] // P)
    cfg["TPAD"] = cfg["TILES"] * P
    cfg["VPAD"] = cfg["NC"] * cfg["TPAD"]
    cfg["CH"] = cfg["TPAD"] // NCHUNK
    cfg["CL1"] = cfg["HEADS"] * cfg["HID"]   # 128
    cfg["CL2"] = cfg["HEADS"] * cfg["OUT"]   # 256
    cfg["R1"] = 256                          # padded table1 row (f16 elems)
    cfg["R2"] = 384                          # padded table2 row
    assert cfg["CL1"] == P and cfg["IN"] == P
    assert cfg["TPAD"] % NCHUNK == 0
    return cfg


def _wrap16(a):
    """int16 index list -> [16, n/16] wrapped layout (idx j at [j%16, j//16])."""
    assert a.size % 16 == 0
    return np.ascontiguousarray(a.reshape(-1, 16).T.astype(np.int16))


# ---------------------------------------------------------------- host prep
def _preprocess(cfg, x, edge_index, batch):
    """Integer/layout-only host preprocessing."""
    NC, N, NLOC, TILES, TPAD = cfg["NC"], cfg["N"], cfg["NLOC"], cfg["TILES"], cfg["TPAD"]
    CH = cfg["CH"]
    src = np.concatenate([edge_index[0].astype(np.int64), np.arange(N, dtype=np.int64)])
    dst = np.concatenate([edge_index[1].astype(np.int64), np.arange(N, dtype=np.int64)])
    order = np.argsort(dst, kind="stable")
    srcs, dsts = src[order], dst[order]

    # chunk-major global row: g(k, loc) = (loc//CH)*(NC*CH) + k*CH + loc%CH
    sk, sl = srcs // NLOC, srcs % NLOC
    gsrc = (sl // CH) * (NC * CH) + sk * CH + (sl % CH)

    core_bounds = np.searchsorted(dsts, np.arange(NC + 1) * NLOC)
    tile_of = ((dsts % NLOC) // P).astype(np.int64)

    # per-(core,tile) A/B counts  (A: gsrc < HALF)
    isA = gsrc < HALF
    nA = np.zeros((NC, TILES), np.int64)
    nB = np.zeros((NC, TILES), np.int64)
    for k in range(NC):
        slc = slice(core_bounds[k], core_bounds[k + 1])
        tl = tile_of[slc]
        a = isA[slc]
        nA[k] = np.bincount(tl[a], minlength=TILES)
        nB[k] = np.bincount(tl[~a], minlength=TILES)
    WA = np.maximum(1, -(-nA.max(axis=0) // P)).astype(np.int64)
    WB = np.maximum(1, -(-nB.max(axis=0) // P)).astype(np.int64)
    C = WA + WB
    CMAX = int(C.max())

    # packed per-tile sections
    idx_off = np.zeros(TILES + 1, np.int64)   # cols in [16, .] int16 param
    dl_off = np.zeros(TILES + 1, np.int64)    # cols in [128, .] f16 param
    for t in range(TILES):
        idx_off[t + 1] = idx_off[t] + (WA[t] + WB[t] + C[t]) * 8
        dl_off[t + 1] = dl_off[t] + C[t]
    SIDX, SC = int(idx_off[-1]), int(dl_off[-1])

    gidx = np.zeros((NC, P, SIDX), np.int16)
    dlocf = np.full((NC, P, SC), 300.0, np.float16)
    for k in range(NC):
        slc = slice(core_bounds[k], core_bounds[k + 1])
        gs, dl_all = gsrc[slc], (dsts[slc] % NLOC).astype(np.int64)
        tl = tile_of[slc]
        a = isA[slc]
        tile_starts = np.searchsorted(tl, np.arange(TILES + 1))
        for t in range(TILES):
            lo, hi = tile_starts[t], tile_starts[t + 1]
            g_t, d_t, a_t = gs[lo:hi], dl_all[lo:hi] - t * P, a[lo:hi]
            wa, wb, c = int(WA[t]), int(WB[t]), int(C[t])
            sA = np.zeros(wa * P, np.int64)
            sB = np.zeros(wb * P, np.int64)
            dloc_slot = np.full(c * P, 300, np.int64)
            dstid = np.zeros(c * P, np.int64)
            na, nb = int(a_t.sum()), int((~a_t).sum())
            sA[:na] = g_t[a_t]
            sB[:nb] = g_t[~a_t] - HALF
            dloc_slot[:na] = d_t[a_t]
            dloc_slot[wa * P:wa * P + nb] = d_t[~a_t]
            dstid[:na] = t * P + d_t[a_t]
            dstid[wa * P:wa * P + nb] = t * P + d_t[~a_t]
            sec = np.concatenate([_wrap16(sA), _wrap16(sB), _wrap16(dstid)], axis=1)
            # 16-row wrapped pattern replicated to all 128 partitions (one
            # copy per Q7 core group)
            gidx[k, :, idx_off[t]:idx_off[t + 1]] = np.tile(sec, (8, 1))
            dlocf[k, :, dl_off[t]:dl_off[t + 1]] = np.where(
                dloc_slot < 300, dloc_slot, 300
            ).reshape(c, P).T.astype(np.float16)

    # pooling segments per (core, tile): runs of equal batch id
    POOLP, NG = cfg["OUT"], cfg["NG"]
    runs = [[[] for _ in range(TILES)] for _ in range(NC)]
    SEG = 1
    for k in range(NC):
        bl = batch[k * NLOC:(k + 1) * NLOC].astype(np.int64)
        for t in range(TILES):
            lo = t * P
            hi = min((t + 1) * P, NLOC)
            seg = bl[lo:hi]
            if len(seg) == 0:
                continue
            cuts = np.flatnonzero(np.diff(seg)) + 1
            edges = np.concatenate([[0], cuts, [len(seg)]])
            for i in range(len(edges) - 1):
                runs[k][t].append((int(seg[edges[i]]), int(edges[i]), int(edges[i + 1])))
            SEG = max(SEG, len(edges) - 1)
    STOT = TILES * SEG
    segmask = np.zeros((NC, TILES, POOLP, SEG * P), np.float16)
    M = np.zeros((NC, POOLP, NG * STOT), np.float16)
    for k in range(NC):
        for t in range(TILES):
            for s, (g, lo, hi) in enumerate(runs[k][t]):
                segmask[k, t, :, s * P + lo:s * P + hi] = 1.0
                M[k, :, g * STOT + t * SEG + s] = 1.0

    # x transposed/padded/cast per core
    xT16 = np.zeros((NC, P, TPAD), np.float16)
    for k in range(NC):
        xl = x[k * NLOC:(k + 1) * NLOC].astype(np.float16)
        xT16[k, :, :NLOC] = xl.T

    meta = dict(WA=[int(w) for w in WA], WB=[int(w) for w in WB],
                C=[int(c) for c in C], CMAX=CMAX,
                idx_off=[int(o) for o in idx_off], dl_off=[int(o) for o in dl_off],
                SIDX=SIDX, SC=SC, SEG=SEG, STOT=STOT)
    per_core = [dict(xT16=xT16[k], gidx=gidx[k], dlocf=dlocf[k],
                     segmask=segmask[k].reshape(TILES * POOLP, SEG * P),
                     Mfull=M[k]) for k in range(NC)]
    return meta, per_core


def _perm(heads, width):
    """new_col j = cc*heads + h  <-  old col h*width + cc"""
    j = np.arange(heads * width)
    return (j % heads) * width + (j // heads)


def _weights_host(cfg, meta, W1, a_s1, a_d1, b1, W2, a_s2, a_d2, b2, Wl, bl):
    HEADS, HID, OUT = cfg["HEADS"], cfg["HID"], cfg["OUT"]
    CL1, CL2, NCLS, NG = cfg["CL1"], cfg["CL2"], cfg["NCLS"], cfg["NG"]
    f16, f32 = np.float16, np.float32
    p1 = _perm(HEADS, HID)    # CL1 interleave
    p2 = _perm(HEADS, OUT)    # CL2 interleave
    A1 = np.zeros((CL1, 8), f16)
    for h in range(HEADS):
        A1[h * HID:(h + 1) * HID, h] = a_s1[h].astype(f16)
        A1[h * HID:(h + 1) * HID, 4 + h] = a_d1[h].astype(f16)
    A2 = np.zeros((CL2, 8), f16)
    for h in range(HEADS):
        A2[h * OUT:(h + 1) * OUT, h] = a_s2[h].astype(f16)
        A2[h * OUT:(h + 1) * OUT, 4 + h] = a_d2[h].astype(f16)
    CMAX = meta["CMAX"]
    sh = dict(
        IOTABIG=np.tile(np.arange(P, dtype=f16)[None, :], (P, CMAX)),
        W1T16=np.ascontiguousarray(W1.T[:, p1].astype(f16)),       # [IN, CL1] interleaved out
        W116=W1.astype(f16).copy(),                                # [CL1, IN] (orig, for B1 fold)
        A1blk=A1,
        W2T16=np.ascontiguousarray(W2.T[p1][:, p2].astype(f16)),   # [CL1(il), CL2(il)]
        W216a=np.ascontiguousarray(W2[:P][:, p1].astype(f16)),     # for B2 fold (u-space il)
        W216b=np.ascontiguousarray(W2[P:][:, p1].astype(f16)),
        A2blka=A2[:P].copy(), A2blkb=A2[P:].copy(),
        b1row=np.tile(b1.astype(f32)[p1][None, :], (P, 1)),        # [128, CL1] interleaved
        b2row=np.tile(b2.astype(f32)[None, :], (P, 1)),            # [128, OUT]
        blrow=np.tile(bl.astype(f32)[None, :], (NG, 1)),           # [NG, NCLS]
        WlT32=Wl.T.astype(f32).copy(),                             # [OUT, NCLS]
    )
    return sh


# ---------------------------------------------------------------- program
def _build(cfg, meta):
    import concourse.bass as bass
    import concourse.bacc as bacc
    import concourse.tile as tile
    from concourse import mybir
    from concourse.masks import make_identity

    f16, f32, i16 = mybir.dt.float16, mybir.dt.float32, mybir.dt.int16
    Alu = mybir.AluOpType
    Act = mybir.ActivationFunctionType
    NC, TILES, TPAD, VPAD, CH = cfg["NC"], cfg["TILES"], cfg["TPAD"], cfg["VPAD"], cfg["CH"]
    CL1, CL2, R1, R2 = cfg["CL1"], cfg["CL2"], cfg["R1"], cfg["R2"]
    OUT, NG, NCLS, NEG = cfg["OUT"], cfg["NG"], cfg["NCLS"], cfg["NEG"]
    HEADS, HID = cfg["HEADS"], cfg["HID"]
    SEG, STOT = meta["SEG"], meta["STOT"]
    WAl, WBl, Cl = meta["WA"], meta["WB"], meta["C"]
    idx_off, dl_off = meta["idx_off"], meta["dl_off"]
    POOLP = OUT
    OFF = 1024.0  # masked-max offset

    nc = bacc.Bacc("TRN2", target_bir_lowering=False, debug=False, num_devices=NC)

    pr = {}
    def param(name, shape, dt, out=False):
        pr[name] = nc.declare_dram_parameter(name, list(shape), dt, isOutput=out)
        return pr[name]

    param("xT16", [P, TPAD], f16)
    param("gidx", [P, meta["SIDX"]], i16)
    param("dlocf", [P, meta["SC"]], f16)
    param("segmask", [TILES * POOLP, SEG * P], f16)
    param("Mfull", [POOLP, NG * STOT], f16)
    param("IOTABIG", [P, meta["CMAX"] * P], f16)
    param("W1T16", [P, CL1], f16)
    param("W116", [CL1, P], f16)
    param("A1blk", [CL1, 8], f16)
    param("W2T16", [CL1, CL2], f16)
    param("W216a", [P, CL1], f16)
    param("W216b", [CL2 - P, CL1], f16)
    param("A2blka", [P, 8], f16)
    param("A2blkb", [CL2 - P, 8], f16)
    param("b1row", [P, CL1], f32)
    param("b2row", [P, OUT], f32)
    param("blrow", [NG, NCLS], f32)
    param("WlT32", [OUT, NCLS], f32)
    out_ext = param("out_logits", [NG, NCLS], f32, out=True)

    rg = [list(range(NC))]

    with tile.TileContext(nc) as tc:
        with tc.tile_pool(name="const", bufs=1) as cp, \
             tc.tile_pool(name="dram", bufs=1, space="DRAM") as dp:

            # ---- constants to SBUF
            def ld(name, shape, dt):
                t_ = cp.tile(list(shape), dt, name="c_" + name)
                nc.sync.dma_start(t_[:], pr[name][:])
                return t_
            xT = ld("xT16", [P, TPAD], f16)
            iotaB = ld("IOTABIG", [P, meta["CMAX"] * P], f16)
            W1T = ld("W1T16", [P, CL1], f16)
            W116 = ld("W116", [CL1, P], f16)
            A1blk = ld("A1blk", [CL1, 8], f16)
            W2T = ld("W2T16", [CL1, CL2], f16)
            W216a = ld("W216a", [P, CL1], f16)
            W216b = ld("W216b", [CL2 - P, CL1], f16)
            A2blka = ld("A2blka", [P, 8], f16)
            A2blkb = ld("A2blkb", [CL2 - P, 8], f16)
            b1row = ld("b1row", [P, CL1], f32)
            b2row = ld("b2row", [P, OUT], f32)
            blrow = ld("blrow", [NG, NCLS], f32)
            WlT32 = ld("WlT32", [OUT, NCLS], f32)
            Mfull = ld("Mfull", [POOLP, NG * STOT], f16)

            ident = cp.tile([P, P], f32)
            make_identity(nc, ident[:])

            # ---- B1/B2 fold matrices (al = x @ B1 ; al2 = u @ B2)
            B1 = cp.tile([P, 8], f16)
            B2 = cp.tile([P, 8], f16)
            with tc.tile_pool(name="cps", bufs=1, space="PSUM") as cps:
                B1ps = cps.tile([P, 8], f32)
                nc.tensor.matmul(B1ps[:], lhsT=W116[:], rhs=A1blk[:], start=True, stop=True)
                nc.vector.tensor_copy(out=B1[:], in_=B1ps[:])
                B2ps = cps.tile([P, 8], f32)
                nc.tensor.matmul(B2ps[:], lhsT=W216a[:], rhs=A2blka[:], start=True, stop=False)
                nc.tensor.matmul(B2ps[:], lhsT=W216b[:], rhs=A2blkb[:], start=False, stop=True)
                nc.vector.tensor_copy(out=B2[:], in_=B2ps[:])

            # ---- DRAM internals (padded rows)
            tb1_loc = dp.tile([TPAD, R1], f16)
            tb1_full = dp.tile([VPAD, R1], f16)
            tb2_loc = dp.tile([TPAD, R2], f16)
            tb2_full = dp.tile([VPAD, R2], f16)
            ad1P = dp.tile([TPAD, P], f16)      # compact 256B rows: [as|ad|junk]
            ad2P = dp.tile([TPAD, P], f16)
            ar_in = dp.tile([POOLP, NG], f32)
            ar_out = dp.tile([POOLP, NG], f32)

            # ================= phase B: table1 build =================
            with tc.tile_pool(name="phB", bufs=3) as pb, \
                 tc.tile_pool(name="phBps", bufs=2, space="PSUM") as pbps:
                for t in range(TILES):
                    xsl = xT[:, t * P:(t + 1) * P]
                    h1ps = pbps.tile([P, CL1], f32, tag="h1ps")
                    nc.tensor.matmul(h1ps[:], lhsT=xsl, rhs=W1T[:], start=True, stop=True)
                    alps = pbps.tile([P, 8], f32, tag="alps")
                    nc.tensor.matmul(alps[:], lhsT=xsl, rhs=B1[:], start=True, stop=True)
                    h116 = pb.tile([P, CL1], f16, tag="h116")
                    nc.vector.tensor_copy(out=h116[:], in_=h1ps[:])
                    al16 = pb.tile([P, 8], f16, tag="al16")
                    nc.vector.tensor_copy(out=al16[:], in_=alps[:])
                    nc.sync.dma_start(tb1_loc[t * P:(t + 1) * P, 0:CL1], h116[:])
                    nc.sync.dma_start(tb1_loc[t * P:(t + 1) * P, CL1:CL1 + 8], al16[:])
                    nc.sync.dma_start(ad1P[t * P:(t + 1) * P, 0:8], al16[:])

            for c in range(NCHUNK):
                nc.gpsimd.collective_compute(
                    "AllGather", mybir.AluOpType.bypass, replica_groups=rg,
                    ins=[tb1_loc[c * CH:(c + 1) * CH, :].opt()],
                    outs=[tb1_full[c * NC * CH:(c + 1) * NC * CH, :].opt()])

            # ================= edge phases =================
            def edge_tile(pe, pp, t, CL, ROWW, tfull, tloc, epilogue):
                RW = CL + 4
                WA, WB, C = WAl[t], WBl[t], Cl[t]
                io, do = idx_off[t], dl_off[t]
                it = pe.tile([P, (2 * C) * 8], i16, tag="it")
                nc.sync.dma_start(it[:], pr["gidx"][:, io:io + 2 * C * 8])
                dlt = pe.tile([P, C], f16, tag="dlt")
                nc.sync.dma_start(dlt[:], pr["dlocf"][:, do:do + C])

                hg = pe.tile([P, C * ROWW], f16, tag="hg")
                adg = pe.tile([P, C * P], f16, tag="adg")
                if STUB_GATHER:
                    nc.vector.memset(hg[:], 0.0)
                    nc.vector.memset(adg[:], 0.0)
                else:
                    nc.gpsimd.dma_gather(
                        out_ap=hg[:, 0:WA * ROWW].rearrange("p (w r) -> p w r", w=WA),
                        in_ap=tfull[0:HALF, :], idxs_ap=it[:, 0:WA * 8],
                        num_idxs=WA * P, num_idxs_reg=WA * P, elem_size=ROWW,
                        single_packet=False)
                    nc.gpsimd.dma_gather(
                        out_ap=hg[:, WA * ROWW:].rearrange("p (w r) -> p w r", w=WB),
                        in_ap=tfull[HALF:VPAD, :], idxs_ap=it[:, WA * 8:C * 8],
                        num_idxs=WB * P, num_idxs_reg=WB * P, elem_size=ROWW,
                        single_packet=False)
                    nc.gpsimd.dma_gather(
                        out_ap=adg[:].rearrange("p (w r) -> p w r", w=C),
                        in_ap=tloc[:], idxs_ap=it[:, C * 8:2 * C * 8],
                        num_idxs=C * P, num_idxs_reg=C * P, elem_size=P,
                        single_packet=False)

                hg3 = hg[:].rearrange("p (w r) -> p w r", w=C)
                adg3 = adg[:].rearrange("p (w r) -> p w r", w=C)

                # mask: expand dloc on Scalar engine, compare on DVE (2x)
                dlE = pe.tile([P, C * P], f16, tag="dlE")
                nc.scalar.activation(
                    dlE[:].rearrange("p (c d) -> p c d", c=C),
                    dlt[:, :, None].to_broadcast([P, C, P]), Act.Copy)
                mask = pe.tile([P, C * P], f16, tag="mask")
                nc.vector.tensor_tensor(out=mask[:], in0=dlE[:],
                                        in1=iotaB[:, 0:C * P], op=Alu.is_equal)

                # attention logits -> ex
                sc = pe.tile([P, C * 4], f32, tag="sc")
                nc.vector.tensor_tensor(
                    out=sc[:].rearrange("p (w h) -> p w h", w=C),
                    in0=hg3[:, :, CL:CL + 4], in1=adg3[:, :, 4:8], op=Alu.add)
                lr = pe.tile([P, C * 4], f32, tag="lr")
                nc.vector.scalar_tensor_tensor(out=lr[:], in0=sc[:], scalar=NEG,
                                               in1=sc[:], op0=Alu.mult, op1=Alu.max)
                ex = pe.tile([P, C * 4], f16, tag="ex")
                nc.scalar.activation(ex[:], lr[:], Act.Exp)
                ex3 = ex[:].rearrange("p (w h) -> p w h", w=C)

                # rhs: head-interleaved h * ex (2x eligible), plus ex columns
                rhs = pe.tile([P, C * RW], f16, tag="rhs")
                rhs3 = rhs[:].rearrange("p (w r) -> p w r", w=C)
                nc.vector.tensor_tensor(
                    out=rhs3[:, :, 0:CL].rearrange("p w (q h) -> p w q h", h=HEADS),
                    in0=hg3[:, :, 0:CL].rearrange("p w (q h) -> p w q h", h=HEADS),
                    in1=ex3[:, :, None, :].to_broadcast([P, C, CL // HEADS, HEADS]),
                    op=Alu.mult)
                nc.vector.tensor_copy(out=rhs3[:, :, CL:CL + 4], in_=ex3)

                num = pp.tile([P, RW], f32, tag="num")
                for j in range(C):
                    nc.tensor.matmul(num[:], lhsT=mask[:, j * P:(j + 1) * P],
                                     rhs=rhs[:, j * RW:(j + 1) * RW],
                                     start=(j == 0), stop=(j == C - 1))
                epilogue(t, num)

            # ---- layer 1 (+ table2 build in epilogue)
            with tc.tile_pool(name="ph1", bufs=2) as p1, \
                 tc.tile_pool(name="ph1b", bufs=2) as p1b, \
                 tc.tile_pool(name="ph1num", bufs=2, space="PSUM") as p1num, \
                 tc.tile_pool(name="ph1ps", bufs=1, space="PSUM") as p1ps:

                def epi1(t, num):
                    den = p1b.tile([P, 4], f32, tag="den")
                    nc.vector.tensor_scalar_add(out=den[:], in0=num[:, CL1:CL1 + 4], scalar1=1e-16)
                    rden = p1b.tile([P, 4], f32, tag="rden")
                    nc.vector.reciprocal(rden[:], den[:])
                    rdE = p1b.tile([P, CL1], f32, tag="rdE")
                    nc.vector.tensor_copy(
                        out=rdE[:].rearrange("p (q h) -> p q h", h=HEADS),
                        in_=rden[:, None, :].to_broadcast([P, CL1 // HEADS, HEADS]))
                    mu = p1b.tile([P, CL1], f32, tag="mu")
                    nc.vector.tensor_tensor(out=mu[:], in0=num[:, 0:CL1], in1=rdE[:],
                                            op=Alu.mult)
                    mb = p1b.tile([P, CL1], f32, tag="mb")
                    nc.vector.tensor_tensor(out=mb[:], in0=mu[:], in1=b1row[:], op=Alu.add)
                    ur = p1b.tile([P, CL1], f32, tag="ur")
                    nc.scalar.activation(ur[:], mb[:], Act.Relu)
                    uTps = p1ps.tile([P, P], f32, tag="uTps")
                    nc.tensor.transpose(out=uTps[:], in_=ur[:], identity=ident[:])
                    uT16 = p1b.tile([P, P], f16, tag="uT16")
                    nc.vector.tensor_copy(out=uT16[:], in_=uTps[:])
                    h2ps = p1ps.tile([P, CL2], f32, tag="h2ps")
                    nc.tensor.matmul(h2ps[:], lhsT=uT16[:], rhs=W2T[:], start=True, stop=True)
                    al2ps = p1ps.tile([P, 8], f32, tag="al2ps")
                    nc.tensor.matmul(al2ps[:], lhsT=uT16[:], rhs=B2[:], start=True, stop=True)
                    h216 = p1b.tile([P, CL2], f16, tag="h216")
                    nc.vector.tensor_copy(out=h216[:], in_=h2ps[:])
                    al216 = p1b.tile([P, 8], f16, tag="al216")
                    nc.vector.tensor_copy(out=al216[:], in_=al2ps[:])
                    nc.sync.dma_start(tb2_loc[t * P:(t + 1) * P, 0:CL2], h216[:])
                    nc.sync.dma_start(tb2_loc[t * P:(t + 1) * P, CL2:CL2 + 8], al216[:])
                    nc.sync.dma_start(ad2P[t * P:(t + 1) * P, 0:8], al216[:])

                for t in range(TILES):
                    edge_tile(p1, p1num, t, CL1, R1, tb1_full, ad1P, epi1)

            for c in range(NCHUNK):
                nc.gpsimd.collective_compute(
                    "AllGather", mybir.AluOpType.bypass, replica_groups=rg,
                    ins=[tb2_loc[c * CH:(c + 1) * CH, :].opt()],
                    outs=[tb2_full[c * NC * CH:(c + 1) * NC * CH, :].opt()])

            # ---- layer 2 + pooling
            partial = cp.tile([POOLP, STOT], f32)
            with tc.tile_pool(name="ph2", bufs=2) as p2, \
                 tc.tile_pool(name="ph2b", bufs=2) as p2b, \
                 tc.tile_pool(name="ph2num", bufs=2, space="PSUM") as p2num, \
                 tc.tile_pool(name="ph2ps", bufs=1, space="PSUM") as p2ps:

                def epi2(t, num):
                    den = p2b.tile([P, 4], f32, tag="den2")
                    nc.vector.tensor_scalar_add(out=den[:], in0=num[:, CL2:CL2 + 4], scalar1=1e-16)
                    rden = p2b.tile([P, 4], f32, tag="rden2")
                    nc.vector.reciprocal(rden[:], den[:])
                    rdE = p2b.tile([P, CL2], f32, tag="rdE2")
                    nc.vector.tensor_copy(
                        out=rdE[:].rearrange("p (q h) -> p q h", h=HEADS),
                        in_=rden[:, None, :].to_broadcast([P, CL2 // HEADS, HEADS]))
                    mo = p2b.tile([P, CL2], f32, tag="mo")
                    nc.vector.tensor_tensor(out=mo[:], in0=num[:, 0:CL2], in1=rdE[:],
                                            op=Alu.mult)
                    hsum = p2b.tile([P, OUT], f32, tag="hsum")
                    nc.vector.reduce_sum(hsum[:],
                                         mo[:].rearrange("p (q h) -> p q h", h=HEADS),
                                         axis=mybir.AxisListType.X)
                    o2 = p2b.tile([P, OUT], f32, tag="o2")
                    nc.vector.scalar_tensor_tensor(
                        out=o2[:], in0=hsum[:], scalar=1.0 / HEADS,
                        in1=b2row[:], op0=Alu.mult, op1=Alu.add)
                    o2Tps = p2ps.tile([OUT, P], f32, tag="o2Tps")
                    nc.tensor.transpose(out=o2Tps[:], in_=o2[:], identity=ident[:])
                    o2T = p2b.tile([OUT, P], f32, tag="o2T")
                    nc.vector.tensor_copy(out=o2T[:], in_=o2Tps[:])
                    sm = p2b.tile([POOLP, SEG * P], f16, tag="sm")
                    nc.sync.dma_start(sm[:], pr["segmask"][t * POOLP:(t + 1) * POOLP, :])
                    for s in range(SEG):
                        tmp = p2b.tile([POOLP, P], f32, tag="ptmp")
                        nc.vector.scalar_tensor_tensor(
                            out=tmp[:], in0=o2T[:], scalar=OFF,
                            in1=sm[:, s * P:(s + 1) * P], op0=Alu.add, op1=Alu.mult)
                        nc.vector.reduce_max(partial[:, t * SEG + s:t * SEG + s + 1],
                                             tmp[:], axis=mybir.AxisListType.X)

                for t in range(TILES):
                    edge_tile(p2, p2num, t, CL2, R2, tb2_full, ad2P, epi2)

            # ---- combine partials -> per-graph max -> AllReduce -> head
            with tc.tile_pool(name="fin", bufs=1) as fp, \
                 tc.tile_pool(name="finps", bufs=1, space="PSUM") as fps:
                tmpg = fp.tile([POOLP, NG * STOT], f32)
                nc.vector.scalar_tensor_tensor(
                    out=tmpg[:], in0=partial[:][:, None, :].to_broadcast([POOLP, NG, STOT]),
                    scalar=0.0, in1=Mfull[:].rearrange("p (g s) -> p g s", g=NG),
                    op0=Alu.add, op1=Alu.mult)
                pooled = fp.tile([POOLP, NG], f32)
                nc.vector.reduce_max(pooled[:],
                                     tmpg[:].rearrange("p (g s) -> p g s", g=NG),
                                     axis=mybir.AxisListType.X)
                pooled2 = fp.tile([POOLP, NG], f32)
                nc.vector.tensor_scalar_add(out=pooled2[:], in0=pooled[:], scalar1=-OFF)
                nc.sync.dma_start(ar_in[:], pooled2[:])
                nc.gpsimd.collective_compute(
                    "AllReduce", mybir.AluOpType.max, replica_groups=rg,
                    ins=[ar_in[:].opt()], outs=[ar_out[:].opt()])
                pooledF = fp.tile([POOLP, NG], f32)
                nc.sync.dma_start(pooledF[:], ar_out[:])
                lps = fps.tile([NG, NCLS], f32)
                nc.tensor.matmul(lps[:], lhsT=pooledF[:], rhs=WlT32[:], start=True, stop=True)
                logits = fp.tile([NG, NCLS], f32)
                nc.vector.tensor_tensor(out=logits[:], in0=lps[:], in1=blrow[:], op=Alu.add)
                nc.sync.dma_start(out_ext[:], logits[:])

    nc.compile()
    return nc


# ---------------------------------------------------------------- runner
def _make_in_maps(cfg, per_core, shared):
    maps = []
    for k in range(cfg["NC"]):
        m = dict(shared)
        m.update(per_core[k])
        maps.append({k_: np.ascontiguousarray(v) for k_, v in m.items()})
    return maps


def _kernel_impl(inputs, trace=False, trace_kwargs=None):
    cfg = _derive(FULL_CFG)
    x = np.asarray(inputs["x"], np.float32)
    edge_index = np.asarray(inputs["edge_index"]).astype(np.int64)
    batch = np.asarray(inputs["batch"]).astype(np.int64)
    meta, per_core = _preprocess(cfg, x, edge_index, batch)
    shared = _weights_host(
        cfg, meta,
        np.asarray(inputs["W1"], np.float32), np.asarray(inputs["a_src1"], np.float32),
        np.asarray(inputs["a_dst1"], np.float32), np.asarray(inputs["b1"], np.float32),
        np.asarray(inputs["W2"], np.float32), np.asarray(inputs["a_src2"], np.float32),
        np.asarray(inputs["a_dst2"], np.float32), np.asarray(inputs["b2"], np.float32),
        np.asarray(inputs["Wl"], np.float32), np.asarray(inputs["bl"], np.float32))
    nc = _build(cfg, meta)
    in_maps = _make_in_maps(cfg, per_core, shared)
    from concourse.bass_utils import run_bass_kernel_spmd
    res = run_bass_kernel_spmd(nc, in_maps, core_ids=list(range(cfg["NC"])),
                               trace=trace, **(trace_kwargs or {}))
    return np.asarray(res.results[0]["out_logits"], np.float32), res


def kernel(**inputs):
    return _kernel_impl(inputs)[0]


# revision 28
# speedup vs baseline: 2.1207x; 1.5585x over previous
"""Trainium2 Bass kernel for a 2-layer GAT + global max pool + linear head.

Contract: kernel(**inputs) takes FULL unsharded inputs (as produced by
reference.setup_inputs) and returns the FULL [N_GRAPHS, N_CLASSES] float32
output. Internally: shards nodes (and their incident edges, 1D partitioned
by destination) across 8 NeuronCores, replicates the small GAT weights,
AllGathers the per-layer node-feature tables (chunked, overlapped with
compute), and AllReduces the pooled per-graph maxima.

Edge gathers use bulk dma_gather (SWDGE) ops: per destination tile, two
source-row gathers (table split at 32768 rows for int16 indices) plus one
destination sub-row gather for the attention-dst coefficients.

Self-contained: hardcodes all shapes; reads nothing from /root/problem.
"""
import sys

sys.path.insert(0, "/opt/trn_rl_repo")
sys.path.insert(0, "/opt/pypackages")

import numpy as np

# ---------------------------------------------------------------- constants
FULL_CFG = dict(
    N=50000, IN=128, HID=32, OUT=64, HEADS=4, NCLS=32, NG=64,
    NC=8, NEG=0.2,
)

P = 128
HALF = 32768          # int16 index limit for dma_gather
NCHUNK = 4            # AllGather chunks per table
STUB_GATHER = False   # debug: replace gathers with memsets


def _derive(cfg):
    cfg = dict(cfg)
    assert cfg["N"] % cfg["NC"] == 0
    cfg["NLOC"] = cfg["N"] // cfg["NC"]
    cfg["TILES"] = -(-cfg["NLOC"] // P)
    cfg["TPAD"] = cfg["TILES"] * P
    cfg["VPAD"] = cfg["NC"] * cfg["TPAD"]
    cfg["CH"] = cfg["TPAD"] // NCHUNK
    cfg["CL1"] = cfg["HEADS"] * cfg["HID"]   # 128
    cfg["CL2"] = cfg["HEADS"] * cfg["OUT"]   # 256
    cfg["R1"] = 256                          # padded table1 row (f16 elems)
    cfg["R2"] = 384                          # padded table2 row
    assert cfg["CL1"] == P and cfg["IN"] == P
    assert cfg["TPAD"] % NCHUNK == 0
    return cfg


def _wrap16(a):
    """int16 index list -> [16, n/16] wrapped layout (idx j at [j%16, j//16])."""
    assert a.size % 16 == 0
    return np.ascontiguousarray(a.reshape(-1, 16).T.astype(np.int16))


# ---------------------------------------------------------------- host prep
def _preprocess(cfg, x, edge_index, batch):
    """Integer/layout-only host preprocessing."""
    NC, N, NLOC, TILES, TPAD = cfg["NC"], cfg["N"], cfg["NLOC"], cfg["TILES"], cfg["TPAD"]
    CH = cfg["CH"]
    src = edge_index[0].astype(np.int64)
    dst = edge_index[1].astype(np.int64)
    order = np.argsort(dst, kind="stable")
    srcs, dsts = src[order], dst[order]

    # chunk-major global row: g(k, loc) = (loc//CH)*(NC*CH) + k*CH + loc%CH
    sk, sl = srcs // NLOC, srcs % NLOC
    gsrc = (sl // CH) * (NC * CH) + sk * CH + (sl % CH)

    core_bounds = np.searchsorted(dsts, np.arange(NC + 1) * NLOC)
    tile_of = ((dsts % NLOC) // P).astype(np.int64)

    # per-(core,tile) A/B counts  (A: gsrc < HALF)
    isA = gsrc < HALF
    nA = np.zeros((NC, TILES), np.int64)
    nB = np.zeros((NC, TILES), np.int64)
    for k in range(NC):
        slc = slice(core_bounds[k], core_bounds[k + 1])
        tl = tile_of[slc]
        a = isA[slc]
        nA[k] = np.bincount(tl[a], minlength=TILES)
        nB[k] = np.bincount(tl[~a], minlength=TILES)
    WA = np.maximum(1, -(-nA.max(axis=0) // P)).astype(np.int64)
    WB = np.maximum(1, -(-nB.max(axis=0) // P)).astype(np.int64)
    C = WA + WB
    CMAX = int(C.max())

    # packed per-tile sections
    idx_off = np.zeros(TILES + 1, np.int64)   # cols in [16, .] int16 param
    dl_off = np.zeros(TILES + 1, np.int64)    # cols in [128, .] f16 param
    for t in range(TILES):
        idx_off[t + 1] = idx_off[t] + (WA[t] + WB[t]) * 8
        dl_off[t + 1] = dl_off[t] + C[t]
    SIDX, SC = int(idx_off[-1]), int(dl_off[-1])

    gidx = np.zeros((NC, P, SIDX), np.int16)
    dlocf = np.full((NC, P, SC), 300.0, np.float16)
    dlocflat = np.full((NC, 1, SC * P), 300.0, np.float16)
    for k in range(NC):
        slc = slice(core_bounds[k], core_bounds[k + 1])
        gs, dl_all = gsrc[slc], (dsts[slc] % NLOC).astype(np.int64)
        tl = tile_of[slc]
        a = isA[slc]
        tile_starts = np.searchsorted(tl, np.arange(TILES + 1))
        for t in range(TILES):
            lo, hi = tile_starts[t], tile_starts[t + 1]
            g_t, d_t, a_t = gs[lo:hi], dl_all[lo:hi] - t * P, a[lo:hi]
            wa, wb, c = int(WA[t]), int(WB[t]), int(C[t])
            sA = np.zeros(wa * P, np.int64)
            sB = np.zeros(wb * P, np.int64)
            dloc_slot = np.full(c * P, 300, np.int64)
            na, nb = int(a_t.sum()), int((~a_t).sum())
            sA[:na] = g_t[a_t]
            sB[:nb] = g_t[~a_t] - HALF
            dloc_slot[:na] = d_t[a_t]
            dloc_slot[wa * P:wa * P + nb] = d_t[~a_t]
            sec = np.concatenate([_wrap16(sA), _wrap16(sB)], axis=1)
            # 16-row wrapped pattern replicated to all 128 partitions (one
            # copy per Q7 core group)
            gidx[k, :, idx_off[t]:idx_off[t + 1]] = np.tile(sec, (8, 1))
            dl16 = np.where(dloc_slot < 300, dloc_slot, 300).astype(np.float16)
            dlocf[k, :, dl_off[t]:dl_off[t + 1]] = dl16.reshape(c, P).T
            dlocflat[k, 0, dl_off[t] * P:dl_off[t + 1] * P] = dl16

    # pooling segments per (core, tile): runs of equal batch id
    POOLP, NG = cfg["OUT"], cfg["NG"]
    runs = [[[] for _ in range(TILES)] for _ in range(NC)]
    SEG = 1
    for k in range(NC):
        bl = batch[k * NLOC:(k + 1) * NLOC].astype(np.int64)
        for t in range(TILES):
            lo = t * P
            hi = min((t + 1) * P, NLOC)
            seg = bl[lo:hi]
            if len(seg) == 0:
                continue
            cuts = np.flatnonzero(np.diff(seg)) + 1
            edges = np.concatenate([[0], cuts, [len(seg)]])
            for i in range(len(edges) - 1):
                runs[k][t].append((int(seg[edges[i]]), int(edges[i]), int(edges[i + 1])))
            SEG = max(SEG, len(edges) - 1)
    STOT = TILES * SEG
    segmask = np.zeros((NC, TILES, POOLP, SEG * P), np.float16)
    M = np.zeros((NC, POOLP, NG * STOT), np.float16)
    for k in range(NC):
        for t in range(TILES):
            for s, (g, lo, hi) in enumerate(runs[k][t]):
                segmask[k, t, :, s * P + lo:s * P + hi] = 1.0
                M[k, :, g * STOT + t * SEG + s] = 1.0

    # x transposed/padded/cast per core
    xT16 = np.zeros((NC, P, TPAD), np.float16)
    for k in range(NC):
        xl = x[k * NLOC:(k + 1) * NLOC].astype(np.float16)
        xT16[k, :, :NLOC] = xl.T

    meta = dict(WA=[int(w) for w in WA], WB=[int(w) for w in WB],
                C=[int(c) for c in C], CMAX=CMAX,
                idx_off=[int(o) for o in idx_off], dl_off=[int(o) for o in dl_off],
                SIDX=SIDX, SC=SC, SEG=SEG, STOT=STOT)
    per_core = [dict(xT16=xT16[k], gidx=gidx[k], dlocf=dlocf[k],
                     dlocflat=dlocflat[k],
                     segmask=segmask[k].reshape(TILES * POOLP, SEG * P),
                     Mfull=M[k]) for k in range(NC)]
    return meta, per_core


def _perm(heads, width):
    """new_col j = cc*heads + h  <-  old col h*width + cc"""
    j = np.arange(heads * width)
    return (j % heads) * width + (j // heads)


def _weights_host(cfg, meta, W1, a_s1, a_d1, b1, W2, a_s2, a_d2, b2, Wl, bl):
    HEADS, HID, OUT = cfg["HEADS"], cfg["HID"], cfg["OUT"]
    CL1, CL2, NCLS, NG = cfg["CL1"], cfg["CL2"], cfg["NCLS"], cfg["NG"]
    f16, f32 = np.float16, np.float32
    p1 = _perm(HEADS, HID)    # CL1 interleave
    p2 = _perm(HEADS, OUT)    # CL2 interleave
    A1 = np.zeros((CL1, 8), f16)
    for h in range(HEADS):
        A1[h * HID:(h + 1) * HID, h] = a_s1[h].astype(f16)
        A1[h * HID:(h + 1) * HID, 4 + h] = a_d1[h].astype(f16)
    A2 = np.zeros((CL2, 8), f16)
    for h in range(HEADS):
        A2[h * OUT:(h + 1) * OUT, h] = a_s2[h].astype(f16)
        A2[h * OUT:(h + 1) * OUT, 4 + h] = a_d2[h].astype(f16)
    CMAX = meta["CMAX"]
    sh = dict(
        IOTABIG=np.tile(np.arange(P, dtype=f16)[None, :], (P, CMAX)),
        IOTACOL=np.arange(P, dtype=f16)[:, None].copy(),
        W1T16=np.ascontiguousarray(W1.T[:, p1].astype(f16)),       # [IN, CL1] interleaved out
        W116=W1.astype(f16).copy(),                                # [CL1, IN] (orig, for B1 fold)
        A1blk=A1,
        W2T16=np.ascontiguousarray(W2.T[p1][:, p2].astype(f16)),   # [CL1(il), CL2(il)]
        W216a=np.ascontiguousarray(W2[:P][:, p1].astype(f16)),     # for B2 fold (u-space il)
        W216b=np.ascontiguousarray(W2[P:][:, p1].astype(f16)),
        A2blka=A2[:P].copy(), A2blkb=A2[P:].copy(),
        b1row=np.tile(b1.astype(f32)[p1][None, :], (P, 1)),        # [128, CL1] interleaved
        b2row=np.tile(b2.astype(f32)[None, :], (P, 1)),            # [128, OUT]
        blrow=np.tile(bl.astype(f32)[None, :], (NG, 1)),           # [NG, NCLS]
        WlT32=Wl.T.astype(f32).copy(),                             # [OUT, NCLS]
    )
    return sh


# ---------------------------------------------------------------- program
def _build(cfg, meta):
    import concourse.bass as bass
    import concourse.bacc as bacc
    import concourse.tile as tile
    from concourse import mybir
    from concourse.masks import make_identity

    f16, f32, i16 = mybir.dt.float16, mybir.dt.float32, mybir.dt.int16
    Alu = mybir.AluOpType
    Act = mybir.ActivationFunctionType
    NC, TILES, TPAD, VPAD, CH = cfg["NC"], cfg["TILES"], cfg["TPAD"], cfg["VPAD"], cfg["CH"]
    CL1, CL2, R1, R2 = cfg["CL1"], cfg["CL2"], cfg["R1"], cfg["R2"]
    OUT, NG, NCLS, NEG = cfg["OUT"], cfg["NG"], cfg["NCLS"], cfg["NEG"]
    HEADS, HID = cfg["HEADS"], cfg["HID"]
    SEG, STOT = meta["SEG"], meta["STOT"]
    WAl, WBl, Cl = meta["WA"], meta["WB"], meta["C"]
    idx_off, dl_off = meta["idx_off"], meta["dl_off"]
    POOLP = OUT
    OFF = 1024.0  # masked-max offset

    nc = bacc.Bacc("TRN2", target_bir_lowering=False, debug=False, num_devices=NC)

    pr = {}
    def param(name, shape, dt, out=False):
        pr[name] = nc.declare_dram_parameter(name, list(shape), dt, isOutput=out)
        return pr[name]

    param("xT16", [P, TPAD], f16)
    param("gidx", [P, meta["SIDX"]], i16)
    param("dlocf", [P, meta["SC"]], f16)
    param("dlocflat", [1, meta["SC"] * P], f16)
    param("segmask", [TILES * POOLP, SEG * P], f16)
    param("Mfull", [POOLP, NG * STOT], f16)
    param("IOTABIG", [P, meta["CMAX"] * P], f16)
    param("IOTACOL", [P, 1], f16)
    param("W1T16", [P, CL1], f16)
    param("W116", [CL1, P], f16)
    param("A1blk", [CL1, 8], f16)
    param("W2T16", [CL1, CL2], f16)
    param("W216a", [P, CL1], f16)
    param("W216b", [CL2 - P, CL1], f16)
    param("A2blka", [P, 8], f16)
    param("A2blkb", [CL2 - P, 8], f16)
    param("b1row", [P, CL1], f32)
    param("b2row", [P, OUT], f32)
    param("blrow", [NG, NCLS], f32)
    param("WlT32", [OUT, NCLS], f32)
    out_ext = param("out_logits", [NG, NCLS], f32, out=True)

    rg = [list(range(NC))]

    with tile.TileContext(nc) as tc:
        with tc.tile_pool(name="const", bufs=1) as cp, \
             tc.tile_pool(name="dram", bufs=1, space="DRAM") as dp:

            # ---- constants to SBUF
            def ld(name, shape, dt):
                t_ = cp.tile(list(shape), dt, name="c_" + name)
                nc.sync.dma_start(t_[:], pr[name][:])
                return t_
            xT = ld("xT16", [P, TPAD], f16)
            iotaB = ld("IOTABIG", [P, meta["CMAX"] * P], f16)
            W1T = ld("W1T16", [P, CL1], f16)
            W116 = ld("W116", [CL1, P], f16)
            A1blk = ld("A1blk", [CL1, 8], f16)
            W2T = ld("W2T16", [CL1, CL2], f16)
            W216a = ld("W216a", [P, CL1], f16)
            W216b = ld("W216b", [CL2 - P, CL1], f16)
            A2blka = ld("A2blka", [P, 8], f16)
            A2blkb = ld("A2blkb", [CL2 - P, 8], f16)
            b1row = ld("b1row", [P, CL1], f32)
            b2row = ld("b2row", [P, OUT], f32)
            blrow = ld("blrow", [NG, NCLS], f32)
            WlT32 = ld("WlT32", [OUT, NCLS], f32)
            Mfull = ld("Mfull", [POOLP, NG * STOT], f16)

            ident = cp.tile([P, P], f32)
            make_identity(nc, ident[:])
            iotaC = ld("IOTACOL", [P, 1], f16)
            onesW = cp.tile([P, meta["CMAX"] * P], f16)
            nc.vector.memset(onesW[:], 1.0)

            # ---- B1/B2 fold matrices (al = x @ B1 ; al2 = u @ B2)
            B1 = cp.tile([P, 8], f16)
            B2 = cp.tile([P, 8], f16)
            with tc.tile_pool(name="cps", bufs=1, space="PSUM") as cps:
                B1ps = cps.tile([P, 8], f32)
                nc.tensor.matmul(B1ps[:], lhsT=W116[:], rhs=A1blk[:], start=True, stop=True)
                nc.vector.tensor_copy(out=B1[:], in_=B1ps[:])
                B2ps = cps.tile([P, 8], f32)
                nc.tensor.matmul(B2ps[:], lhsT=W216a[:], rhs=A2blka[:], start=True, stop=False)
                nc.tensor.matmul(B2ps[:], lhsT=W216b[:], rhs=A2blkb[:], start=False, stop=True)
                nc.vector.tensor_copy(out=B2[:], in_=B2ps[:])

            # ---- DRAM internals (padded rows)
            tb1_loc = dp.tile([TPAD, R1], f16)
            tb1_full = dp.tile([VPAD, R1], f16)
            tb2_loc = dp.tile([TPAD, R2], f16)
            tb2_full = dp.tile([VPAD, R2], f16)
            ad1P = dp.tile([TPAD, P], f16)      # compact 256B rows: [as|ad|junk]
            ad2P = dp.tile([TPAD, P], f16)
            ar_in = dp.tile([POOLP, NG], f32)
            ar_out = dp.tile([POOLP, NG], f32)

            # ================= phase B: table1 build =================
            with tc.tile_pool(name="phB", bufs=3) as pb, \
                 tc.tile_pool(name="phBps", bufs=2, space="PSUM") as pbps:
                for t in range(TILES):
                    xsl = xT[:, t * P:(t + 1) * P]
                    h1ps = pbps.tile([P, CL1], f32, tag="h1ps")
                    nc.tensor.matmul(h1ps[:], lhsT=xsl, rhs=W1T[:], start=True, stop=True)
                    alps = pbps.tile([P, 8], f32, tag="alps")
                    nc.tensor.matmul(alps[:], lhsT=xsl, rhs=B1[:], start=True, stop=True)
                    h116 = pb.tile([P, CL1], f16, tag="h116")
                    nc.vector.tensor_copy(out=h116[:], in_=h1ps[:])
                    al16 = pb.tile([P, 8], f16, tag="al16")
                    nc.vector.tensor_copy(out=al16[:], in_=alps[:])
                    nc.sync.dma_start(tb1_loc[t * P:(t + 1) * P, 0:CL1], h116[:])
                    nc.sync.dma_start(tb1_loc[t * P:(t + 1) * P, CL1:CL1 + 8], al16[:])
                    nc.sync.dma_start(ad1P[t * P:(t + 1) * P, 0:8], al16[:])

            for c in range(NCHUNK):
                nc.gpsimd.collective_compute(
                    "AllGather", mybir.AluOpType.bypass, replica_groups=rg,
                    ins=[tb1_loc[c * CH:(c + 1) * CH, :].opt()],
                    outs=[tb1_full[c * NC * CH:(c + 1) * NC * CH, :].opt()])

            # ================= edge phases =================
            def edge_tile(pe, pp, ppad, t, CL, ROWW, tfull, tfull_loc, tloc, epilogue):
                RW = CL + 4
                WA, WB, C = WAl[t], WBl[t], Cl[t]
                io, do = idx_off[t], dl_off[t]
                it = pe.tile([P, C * 8], i16, tag="it")
                nc.sync.dma_start(it[:], pr["gidx"][:, io:io + C * 8])
                dlt = pe.tile([P, C], f16, tag="dlt")
                nc.sync.dma_start(dlt[:], pr["dlocf"][:, do:do + C])
                dlR = pe.tile([P, C * P], f16, tag="dlR")
                nc.sync.dma_start(
                    dlR[:], pr["dlocflat"][0:1, do * P:(do + C) * P]
                    .to_broadcast([P, C * P]))
                adt = pe.tile([P, 8], f16, tag="adt")
                nc.sync.dma_start(adt[:], tloc[t * P:(t + 1) * P, 0:8])
                hloc = pe.tile([P, CL], f16, tag="hloc")
                nc.sync.dma_start(hloc[:], tfull_loc[t * P:(t + 1) * P, 0:CL])

                hg = pe.tile([P, C * ROWW], f16, tag="hg")
                nc.gpsimd.dma_gather(
                    out_ap=hg[:, 0:WA * ROWW].rearrange("p (w r) -> p w r", w=WA),
                    in_ap=tfull[0:HALF, :], idxs_ap=it[:, 0:WA * 8],
                    num_idxs=WA * P, num_idxs_reg=WA * P, elem_size=ROWW,
                    single_packet=False)
                nc.gpsimd.dma_gather(
                    out_ap=hg[:, WA * ROWW:].rearrange("p (w r) -> p w r", w=WB),
                    in_ap=tfull[HALF:VPAD, :], idxs_ap=it[:, WA * 8:C * 8],
                    num_idxs=WB * P, num_idxs_reg=WB * P, elem_size=ROWW,
                    single_packet=False)

                hg3 = hg[:].rearrange("p (w r) -> p w r", w=C)

                # slot-major mask: expand dloc on Scalar engine, compare on DVE
                dlE = pe.tile([P, C * P], f16, tag="dlE")
                nc.scalar.activation(
                    dlE[:].rearrange("p (c d) -> p c d", c=C),
                    dlt[:, :, None].to_broadcast([P, C, P]), Act.Copy)
                mask = pe.tile([P, C * P], f16, tag="mask")
                nc.vector.tensor_tensor(out=mask[:], in0=dlE[:],
                                        in1=iotaB[:, 0:C * P], op=Alu.is_equal)

                # transposed mask (partitions = dst) for the ad lookup
                maskT = pe.tile([P, C * P], f16, tag="maskT")
                nc.vector.scalar_tensor_tensor(
                    out=maskT[:], in0=dlR[:], scalar=iotaC[:, 0:1],
                    in1=onesW[:, 0:C * P], op0=Alu.is_equal, op1=Alu.mult)
                adps = ppad.tile([P, C * 8], f32, tag="adps")
                for j in range(C):
                    nc.tensor.matmul(adps[:, j * 8:(j + 1) * 8],
                                     lhsT=maskT[:, j * P:(j + 1) * P],
                                     rhs=adt[:], start=True, stop=True)
                adps3 = adps[:].rearrange("p (w r) -> p w r", w=C)

                # attention logits -> ex
                sc = pe.tile([P, C * 4], f32, tag="sc")
                nc.vector.tensor_tensor(
                    out=sc[:].rearrange("p (w h) -> p w h", w=C),
                    in0=hg3[:, :, CL:CL + 4], in1=adps3[:, :, 4:8], op=Alu.add)
                lr = pe.tile([P, C * 4], f32, tag="lr")
                nc.vector.scalar_tensor_tensor(out=lr[:], in0=sc[:], scalar=NEG,
                                               in1=sc[:], op0=Alu.mult, op1=Alu.max)
                ex = pe.tile([P, C * 4], f16, tag="ex")
                nc.scalar.activation(ex[:], lr[:], Act.Exp)
                ex3 = ex[:].rearrange("p (w h) -> p w h", w=C)

                # rhs: head-interleaved h * ex (2x eligible), plus ex columns
                rhs = pe.tile([P, C * RW], f16, tag="rhs")
                rhs3 = rhs[:].rearrange("p (w r) -> p w r", w=C)
                nc.vector.tensor_tensor(
                    out=rhs3[:, :, 0:CL].rearrange("p w (q h) -> p w q h", h=HEADS),
                    in0=hg3[:, :, 0:CL].rearrange("p w (q h) -> p w q h", h=HEADS),
                    in1=ex3[:, :, None, :].to_broadcast([P, C, CL // HEADS, HEADS]),
                    op=Alu.mult)
                nc.vector.tensor_copy(out=rhs3[:, :, CL:CL + 4], in_=ex3)

                # self-loop contribution from local rows (no gather)
                scS = pe.tile([P, 4], f32, tag="scS")
                nc.vector.tensor_tensor(out=scS[:], in0=adt[:, 0:4],
                                        in1=adt[:, 4:8], op=Alu.add)
                lrS = pe.tile([P, 4], f32, tag="lrS")
                nc.vector.scalar_tensor_tensor(out=lrS[:], in0=scS[:], scalar=NEG,
                                               in1=scS[:], op0=Alu.mult, op1=Alu.max)
                exS = pe.tile([P, 4], f16, tag="exS")
                nc.scalar.activation(exS[:], lrS[:], Act.Exp)
                rhsS = pe.tile([P, RW], f32, tag="rhsS")
                nc.vector.tensor_tensor(
                    out=rhsS[:, 0:CL].rearrange("p (q h) -> p q h", h=HEADS),
                    in0=hloc[:].rearrange("p (q h) -> p q h", h=HEADS),
                    in1=exS[:, None, :].to_broadcast([P, CL // HEADS, HEADS]),
                    op=Alu.mult)
                nc.vector.tensor_copy(out=rhsS[:, CL:CL + 4], in_=exS[:])

                num = pp.tile([P, RW], f32, tag="num")
                for j in range(C):
                    nc.tensor.matmul(num[:], lhsT=mask[:, j * P:(j + 1) * P],
                                     rhs=rhs[:, j * RW:(j + 1) * RW],
                                     start=(j == 0), stop=(j == C - 1))
                nc.vector.tensor_tensor(out=num[:], in0=num[:], in1=rhsS[:],
                                        op=Alu.add)
                epilogue(t, num)

            # ---- layer 1 (+ table2 build in epilogue)
            with tc.tile_pool(name="ph1", bufs=2) as p1, \
                 tc.tile_pool(name="ph1b", bufs=2) as p1b, \
                 tc.tile_pool(name="ph1num", bufs=2, space="PSUM") as p1num, \
                 tc.tile_pool(name="ph1ad", bufs=2, space="PSUM") as p1ad, \
                 tc.tile_pool(name="ph1ps", bufs=1, space="PSUM") as p1ps:

                def epi1(t, num):
                    den = p1b.tile([P, 4], f32, tag="den")
                    nc.vector.tensor_scalar_add(out=den[:], in0=num[:, CL1:CL1 + 4], scalar1=1e-16)
                    rden = p1b.tile([P, 4], f32, tag="rden")
                    nc.vector.reciprocal(rden[:], den[:])
                    rdE = p1b.tile([P, CL1], f32, tag="rdE")
                    nc.vector.tensor_copy(
                        out=rdE[:].rearrange("p (q h) -> p q h", h=HEADS),
                        in_=rden[:, None, :].to_broadcast([P, CL1 // HEADS, HEADS]))
                    mu = p1b.tile([P, CL1], f32, tag="mu")
                    nc.vector.tensor_tensor(out=mu[:], in0=num[:, 0:CL1], in1=rdE[:],
                                            op=Alu.mult)
                    mb = p1b.tile([P, CL1], f32, tag="mb")
                    nc.vector.tensor_tensor(out=mb[:], in0=mu[:], in1=b1row[:], op=Alu.add)
                    ur = p1b.tile([P, CL1], f32, tag="ur")
                    nc.scalar.activation(ur[:], mb[:], Act.Relu)
                    uTps = p1ps.tile([P, P], f32, tag="uTps")
                    nc.tensor.transpose(out=uTps[:], in_=ur[:], identity=ident[:])
                    uT16 = p1b.tile([P, P], f16, tag="uT16")
                    nc.vector.tensor_copy(out=uT16[:], in_=uTps[:])
                    h2ps = p1ps.tile([P, CL2], f32, tag="h2ps")
                    nc.tensor.matmul(h2ps[:], lhsT=uT16[:], rhs=W2T[:], start=True, stop=True)
                    al2ps = p1ps.tile([P, 8], f32, tag="al2ps")
                    nc.tensor.matmul(al2ps[:], lhsT=uT16[:], rhs=B2[:], start=True, stop=True)
                    h216 = p1b.tile([P, CL2], f16, tag="h216")
                    nc.vector.tensor_copy(out=h216[:], in_=h2ps[:])
                    al216 = p1b.tile([P, 8], f16, tag="al216")
                    nc.vector.tensor_copy(out=al216[:], in_=al2ps[:])
                    nc.sync.dma_start(tb2_loc[t * P:(t + 1) * P, 0:CL2], h216[:])
                    nc.sync.dma_start(tb2_loc[t * P:(t + 1) * P, CL2:CL2 + 8], al216[:])
                    nc.sync.dma_start(ad2P[t * P:(t + 1) * P, 0:8], al216[:])

                for t in range(TILES):
                    edge_tile(p1, p1num, p1ad, t, CL1, R1, tb1_full, tb1_loc, ad1P, epi1)

            for c in range(NCHUNK):
                nc.gpsimd.collective_compute(
                    "AllGather", mybir.AluOpType.bypass, replica_groups=rg,
                    ins=[tb2_loc[c * CH:(c + 1) * CH, :].opt()],
                    outs=[tb2_full[c * NC * CH:(c + 1) * NC * CH, :].opt()])

            # ---- layer 2 + pooling
            partial = cp.tile([POOLP, STOT], f32)
            with tc.tile_pool(name="ph2", bufs=2) as p2, \
                 tc.tile_pool(name="ph2b", bufs=2) as p2b, \
                 tc.tile_pool(name="ph2num", bufs=2, space="PSUM") as p2num, \
                 tc.tile_pool(name="ph2ad", bufs=2, space="PSUM") as p2ad, \
                 tc.tile_pool(name="ph2ps", bufs=1, space="PSUM") as p2ps:

                def epi2(t, num):
                    den = p2b.tile([P, 4], f32, tag="den2")
                    nc.vector.tensor_scalar_add(out=den[:], in0=num[:, CL2:CL2 + 4], scalar1=1e-16)
                    rden = p2b.tile([P, 4], f32, tag="rden2")
                    nc.vector.reciprocal(rden[:], den[:])
                    rdE = p2b.tile([P, CL2], f32, tag="rdE2")
                    nc.vector.tensor_copy(
                        out=rdE[:].rearrange("p (q h) -> p q h", h=HEADS),
                        in_=rden[:, None, :].to_broadcast([P, CL2 // HEADS, HEADS]))
                    mo = p2b.tile([P, CL2], f32, tag="mo")
                    nc.vector.tensor_tensor(out=mo[:], in0=num[:, 0:CL2], in1=rdE[:],
                                            op=Alu.mult)
                    hsum = p2b.tile([P, OUT], f32, tag="hsum")
                    nc.vector.reduce_sum(hsum[:],
                                         mo[:].rearrange("p (q h) -> p q h", h=HEADS),
                                         axis=mybir.AxisListType.X)
                    o2 = p2b.tile([P, OUT], f32, tag="o2")
                    nc.vector.scalar_tensor_tensor(
                        out=o2[:], in0=hsum[:], scalar=1.0 / HEADS,
                        in1=b2row[:], op0=Alu.mult, op1=Alu.add)
                    o2Tps = p2ps.tile([OUT, P], f32, tag="o2Tps")
                    nc.tensor.transpose(out=o2Tps[:], in_=o2[:], identity=ident[:])
                    o2T = p2b.tile([OUT, P], f32, tag="o2T")
                    nc.vector.tensor_copy(out=o2T[:], in_=o2Tps[:])
                    sm = p2b.tile([POOLP, SEG * P], f16, tag="sm")
                    nc.sync.dma_start(sm[:], pr["segmask"][t * POOLP:(t + 1) * POOLP, :])
                    for s in range(SEG):
                        tmp = p2b.tile([POOLP, P], f32, tag="ptmp")
                        nc.vector.scalar_tensor_tensor(
                            out=tmp[:], in0=o2T[:], scalar=OFF,
                            in1=sm[:, s * P:(s + 1) * P], op0=Alu.add, op1=Alu.mult)
                        nc.vector.reduce_max(partial[:, t * SEG + s:t * SEG + s + 1],
                                             tmp[:], axis=mybir.AxisListType.X)

                for t in range(TILES):
                    edge_tile(p2, p2num, p2ad, t, CL2, R2, tb2_full, tb2_loc, ad2P, epi2)

            # ---- combine partials -> per-graph max -> AllReduce -> head
            with tc.tile_pool(name="fin", bufs=1) as fp, \
                 tc.tile_pool(name="finps", bufs=1, space="PSUM") as fps:
                tmpg = fp.tile([POOLP, NG * STOT], f32)
                nc.vector.scalar_tensor_tensor(
                    out=tmpg[:], in0=partial[:][:, None, :].to_broadcast([POOLP, NG, STOT]),
                    scalar=0.0, in1=Mfull[:].rearrange("p (g s) -> p g s", g=NG),
                    op0=Alu.add, op1=Alu.mult)
                pooled = fp.tile([POOLP, NG], f32)
                nc.vector.reduce_max(pooled[:],
                                     tmpg[:].rearrange("p (g s) -> p g s", g=NG),
                                     axis=mybir.AxisListType.X)
                pooled2 = fp.tile([POOLP, NG], f32)
                nc.vector.tensor_scalar_add(out=pooled2[:], in0=pooled[:], scalar1=-OFF)
                nc.sync.dma_start(ar_in[:], pooled2[:])
                nc.gpsimd.collective_compute(
                    "AllReduce", mybir.AluOpType.max, replica_groups=rg,
                    ins=[ar_in[:].opt()], outs=[ar_out[:].opt()])
                pooledF = fp.tile([POOLP, NG], f32)
                nc.sync.dma_start(pooledF[:], ar_out[:])
                lps = fps.tile([NG, NCLS], f32)
                nc.tensor.matmul(lps[:], lhsT=pooledF[:], rhs=WlT32[:], start=True, stop=True)
                logits = fp.tile([NG, NCLS], f32)
                nc.vector.tensor_tensor(out=logits[:], in0=lps[:], in1=blrow[:], op=Alu.add)
                nc.sync.dma_start(out_ext[:], logits[:])

    nc.compile()
    return nc


# ---------------------------------------------------------------- runner
def _make_in_maps(cfg, per_core, shared):
    maps = []
    for k in range(cfg["NC"]):
        m = dict(shared)
        m.update(per_core[k])
        maps.append({k_: np.ascontiguousarray(v) for k_, v in m.items()})
    return maps


def _kernel_impl(inputs, trace=False, trace_kwargs=None):
    cfg = _derive(FULL_CFG)
    x = np.asarray(inputs["x"], np.float32)
    edge_index = np.asarray(inputs["edge_index"]).astype(np.int64)
    batch = np.asarray(inputs["batch"]).astype(np.int64)
    meta, per_core = _preprocess(cfg, x, edge_index, batch)
    shared = _weights_host(
        cfg, meta,
        np.asarray(inputs["W1"], np.float32), np.asarray(inputs["a_src1"], np.float32),
        np.asarray(inputs["a_dst1"], np.float32), np.asarray(inputs["b1"], np.float32),
        np.asarray(inputs["W2"], np.float32), np.asarray(inputs["a_src2"], np.float32),
        np.asarray(inputs["a_dst2"], np.float32), np.asarray(inputs["b2"], np.float32),
        np.asarray(inputs["Wl"], np.float32), np.asarray(inputs["bl"], np.float32))
    nc = _build(cfg, meta)
    in_maps = _make_in_maps(cfg, per_core, shared)
    from concourse.bass_utils import run_bass_kernel_spmd
    res = run_bass_kernel_spmd(nc, in_maps, core_ids=list(range(cfg["NC"])),
                               trace=trace, **(trace_kwargs or {}))
    return np.asarray(res.results[0]["out_logits"], np.float32), res


def kernel(**inputs):
    return _kernel_impl(inputs)[0]


# revision 30
# speedup vs baseline: 2.4134x; 1.1381x over previous
"""Trainium2 Bass kernel for a 2-layer GAT + global max pool + linear head.

Contract: kernel(**inputs) takes FULL unsharded inputs (as produced by
reference.setup_inputs) and returns the FULL [N_GRAPHS, N_CLASSES] float32
output. Internally: shards nodes (and their incident edges, 1D partitioned
by destination) across 8 NeuronCores, replicates the small GAT weights,
AllGathers the per-layer node-feature tables (chunked, overlapped with
compute), and AllReduces the pooled per-graph maxima.

Edge gathers use bulk dma_gather (SWDGE) ops: per destination tile, two
source-row gathers (table split at 32768 rows for int16 indices) plus one
destination sub-row gather for the attention-dst coefficients.

Self-contained: hardcodes all shapes; reads nothing from /root/problem.
"""
import sys

sys.path.insert(0, "/opt/trn_rl_repo")
sys.path.insert(0, "/opt/pypackages")

import numpy as np

# ---------------------------------------------------------------- constants
FULL_CFG = dict(
    N=50000, IN=128, HID=32, OUT=64, HEADS=4, NCLS=32, NG=64,
    NC=8, NEG=0.2,
)

P = 128
HALF = 32768          # int16 index limit for dma_gather
NCHUNK = 4            # AllGather chunks per table
STUB_GATHER = False   # debug: replace gathers with memsets


def _derive(cfg):
    cfg = dict(cfg)
    assert cfg["N"] % cfg["NC"] == 0
    cfg["NLOC"] = cfg["N"] // cfg["NC"]
    cfg["TILES"] = -(-cfg["NLOC"] // P)
    cfg["TPAD"] = cfg["TILES"] * P
    cfg["VPAD"] = cfg["NC"] * cfg["TPAD"]
    cfg["CH"] = cfg["TPAD"] // NCHUNK
    cfg["CL1"] = cfg["HEADS"] * cfg["HID"]   # 128
    cfg["CL2"] = cfg["HEADS"] * cfg["OUT"]   # 256
    cfg["R1"] = 256                          # padded table1 row (f16 elems)
    cfg["R2"] = 384                          # padded table2 row
    assert cfg["CL1"] == P and cfg["IN"] == P
    assert cfg["TPAD"] % NCHUNK == 0
    return cfg


def _wrap16(a):
    """int16 index list -> [16, n/16] wrapped layout (idx j at [j%16, j//16])."""
    assert a.size % 16 == 0
    return np.ascontiguousarray(a.reshape(-1, 16).T.astype(np.int16))


# ---------------------------------------------------------------- host prep
def _preprocess(cfg, x, edge_index, batch):
    """Integer/layout-only host preprocessing."""
    NC, N, NLOC, TILES, TPAD = cfg["NC"], cfg["N"], cfg["NLOC"], cfg["TILES"], cfg["TPAD"]
    CH = cfg["CH"]
    src = edge_index[0].astype(np.int64)
    dst = edge_index[1].astype(np.int64)
    order = np.argsort(dst, kind="stable")
    srcs, dsts = src[order], dst[order]

    # chunk-major global row: g(k, loc) = (loc//CH)*(NC*CH) + k*CH + loc%CH
    sk, sl = srcs // NLOC, srcs % NLOC
    gsrc = (sl // CH) * (NC * CH) + sk * CH + (sl % CH)

    core_bounds = np.searchsorted(dsts, np.arange(NC + 1) * NLOC)
    tile_of = ((dsts % NLOC) // P).astype(np.int64)

    # per-(core,tile) A/B counts  (A: gsrc < HALF)
    isA = gsrc < HALF
    nA = np.zeros((NC, TILES), np.int64)
    nB = np.zeros((NC, TILES), np.int64)
    for k in range(NC):
        slc = slice(core_bounds[k], core_bounds[k + 1])
        tl = tile_of[slc]
        a = isA[slc]
        nA[k] = np.bincount(tl[a], minlength=TILES)
        nB[k] = np.bincount(tl[~a], minlength=TILES)
    WA = np.maximum(1, -(-nA.max(axis=0) // P)).astype(np.int64)
    WB = np.maximum(1, -(-nB.max(axis=0) // P)).astype(np.int64)
    C = WA + WB
    CMAX = int(C.max())

    # packed per-tile sections
    idx_off = np.zeros(TILES + 1, np.int64)   # cols in [16, .] int16 param
    dl_off = np.zeros(TILES + 1, np.int64)    # cols in [128, .] f16 param
    for t in range(TILES):
        idx_off[t + 1] = idx_off[t] + (WA[t] + WB[t]) * 8
        dl_off[t + 1] = dl_off[t] + C[t]
    SIDX, SC = int(idx_off[-1]), int(dl_off[-1])

    gidx = np.zeros((NC, P, SIDX), np.int16)
    dlocf = np.full((NC, P, SC), 300.0, np.float16)
    dlocflat = np.full((NC, P, SC * P), 300.0, np.float16)
    for k in range(NC):
        slc = slice(core_bounds[k], core_bounds[k + 1])
        gs, dl_all = gsrc[slc], (dsts[slc] % NLOC).astype(np.int64)
        tl = tile_of[slc]
        a = isA[slc]
        tile_starts = np.searchsorted(tl, np.arange(TILES + 1))
        for t in range(TILES):
            lo, hi = tile_starts[t], tile_starts[t + 1]
            g_t, d_t, a_t = gs[lo:hi], dl_all[lo:hi] - t * P, a[lo:hi]
            wa, wb, c = int(WA[t]), int(WB[t]), int(C[t])
            sA = np.zeros(wa * P, np.int64)
            sB = np.zeros(wb * P, np.int64)
            dloc_slot = np.full(c * P, 300, np.int64)
            na, nb = int(a_t.sum()), int((~a_t).sum())
            sA[:na] = g_t[a_t]
            sB[:nb] = g_t[~a_t] - HALF
            dloc_slot[:na] = d_t[a_t]
            dloc_slot[wa * P:wa * P + nb] = d_t[~a_t]
            sec = np.concatenate([_wrap16(sA), _wrap16(sB)], axis=1)
            # 16-row wrapped pattern replicated to all 128 partitions (one
            # copy per Q7 core group)
            gidx[k, :, idx_off[t]:idx_off[t + 1]] = np.tile(sec, (8, 1))
            dl16 = np.where(dloc_slot < 300, dloc_slot, 300).astype(np.float16)
            dlocf[k, :, dl_off[t]:dl_off[t + 1]] = dl16.reshape(c, P).T
            dlocflat[k, :, dl_off[t] * P:dl_off[t + 1] * P] = dl16[None, :]

    # pooling segments per (core, tile): runs of equal batch id
    POOLP, NG = cfg["OUT"], cfg["NG"]
    runs = [[[] for _ in range(TILES)] for _ in range(NC)]
    SEG = 1
    for k in range(NC):
        bl = batch[k * NLOC:(k + 1) * NLOC].astype(np.int64)
        for t in range(TILES):
            lo = t * P
            hi = min((t + 1) * P, NLOC)
            seg = bl[lo:hi]
            if len(seg) == 0:
                continue
            cuts = np.flatnonzero(np.diff(seg)) + 1
            edges = np.concatenate([[0], cuts, [len(seg)]])
            for i in range(len(edges) - 1):
                runs[k][t].append((int(seg[edges[i]]), int(edges[i]), int(edges[i + 1])))
            SEG = max(SEG, len(edges) - 1)
    STOT = TILES * SEG
    segmask = np.zeros((NC, TILES, POOLP, SEG * P), np.float16)
    M = np.zeros((NC, POOLP, NG * STOT), np.float16)
    for k in range(NC):
        for t in range(TILES):
            for s, (g, lo, hi) in enumerate(runs[k][t]):
                segmask[k, t, :, s * P + lo:s * P + hi] = 1.0
                M[k, :, g * STOT + t * SEG + s] = 1.0

    # x transposed/padded/cast per core
    xT16 = np.zeros((NC, P, TPAD), np.float16)
    for k in range(NC):
        xl = x[k * NLOC:(k + 1) * NLOC].astype(np.float16)
        xT16[k, :, :NLOC] = xl.T

    meta = dict(WA=[int(w) for w in WA], WB=[int(w) for w in WB],
                C=[int(c) for c in C], CMAX=CMAX,
                idx_off=[int(o) for o in idx_off], dl_off=[int(o) for o in dl_off],
                SIDX=SIDX, SC=SC, SEG=SEG, STOT=STOT)
    per_core = [dict(xT16=xT16[k], gidx=gidx[k], dlocf=dlocf[k],
                     dlocflat=dlocflat[k],
                     segmask=segmask[k].reshape(TILES * POOLP, SEG * P),
                     Mfull=M[k]) for k in range(NC)]
    return meta, per_core


def _perm(heads, width):
    """new_col j = cc*heads + h  <-  old col h*width + cc"""
    j = np.arange(heads * width)
    return (j % heads) * width + (j // heads)


def _weights_host(cfg, meta, W1, a_s1, a_d1, b1, W2, a_s2, a_d2, b2, Wl, bl):
    HEADS, HID, OUT = cfg["HEADS"], cfg["HID"], cfg["OUT"]
    CL1, CL2, NCLS, NG = cfg["CL1"], cfg["CL2"], cfg["NCLS"], cfg["NG"]
    f16, f32 = np.float16, np.float32
    p1 = _perm(HEADS, HID)    # CL1 interleave
    p2 = _perm(HEADS, OUT)    # CL2 interleave
    A1 = np.zeros((CL1, 8), f16)
    for h in range(HEADS):
        A1[h * HID:(h + 1) * HID, h] = a_s1[h].astype(f16)
        A1[h * HID:(h + 1) * HID, 4 + h] = a_d1[h].astype(f16)
    A2 = np.zeros((CL2, 8), f16)
    for h in range(HEADS):
        A2[h * OUT:(h + 1) * OUT, h] = a_s2[h].astype(f16)
        A2[h * OUT:(h + 1) * OUT, 4 + h] = a_d2[h].astype(f16)
    CMAX = meta["CMAX"]
    sh = dict(
        IOTABIG=np.tile(np.arange(P, dtype=f16)[None, :], (P, CMAX)),
        IOTACOL=np.arange(P, dtype=f16)[:, None].copy(),
        W1T16=np.ascontiguousarray(W1.T[:, p1].astype(f16)),       # [IN, CL1] interleaved out
        W116=W1.astype(f16).copy(),                                # [CL1, IN] (orig, for B1 fold)
        A1blk=A1,
        W2T16=np.ascontiguousarray(W2.T[p1][:, p2].astype(f16)),   # [CL1(il), CL2(il)]
        W216a=np.ascontiguousarray(W2[:P][:, p1].astype(f16)),     # for B2 fold (u-space il)
        W216b=np.ascontiguousarray(W2[P:][:, p1].astype(f16)),
        A2blka=A2[:P].copy(), A2blkb=A2[P:].copy(),
        b1row=np.tile(b1.astype(f32)[p1][None, :], (P, 1)),        # [128, CL1] interleaved
        b2row=np.tile(b2.astype(f32)[None, :], (P, 1)),            # [128, OUT]
        blrow=np.tile(bl.astype(f32)[None, :], (NG, 1)),           # [NG, NCLS]
        WlT32=Wl.T.astype(f32).copy(),                             # [OUT, NCLS]
    )
    return sh


# ---------------------------------------------------------------- program
def _build(cfg, meta):
    import concourse.bass as bass
    import concourse.bacc as bacc
    import concourse.tile as tile
    from concourse import mybir
    from concourse.masks import make_identity

    f16, f32, i16 = mybir.dt.float16, mybir.dt.float32, mybir.dt.int16
    Alu = mybir.AluOpType
    Act = mybir.ActivationFunctionType
    NC, TILES, TPAD, VPAD, CH = cfg["NC"], cfg["TILES"], cfg["TPAD"], cfg["VPAD"], cfg["CH"]
    CL1, CL2, R1, R2 = cfg["CL1"], cfg["CL2"], cfg["R1"], cfg["R2"]
    OUT, NG, NCLS, NEG = cfg["OUT"], cfg["NG"], cfg["NCLS"], cfg["NEG"]
    HEADS, HID = cfg["HEADS"], cfg["HID"]
    SEG, STOT = meta["SEG"], meta["STOT"]
    WAl, WBl, Cl = meta["WA"], meta["WB"], meta["C"]
    idx_off, dl_off = meta["idx_off"], meta["dl_off"]
    POOLP = OUT
    OFF = 1024.0  # masked-max offset

    nc = bacc.Bacc("TRN2", target_bir_lowering=False, debug=False, num_devices=NC)

    pr = {}
    def param(name, shape, dt, out=False):
        pr[name] = nc.declare_dram_parameter(name, list(shape), dt, isOutput=out)
        return pr[name]

    param("xT16", [P, TPAD], f16)
    param("gidx", [P, meta["SIDX"]], i16)
    param("dlocf", [P, meta["SC"]], f16)
    param("dlocflat", [P, meta["SC"] * P], f16)
    param("segmask", [TILES * POOLP, SEG * P], f16)
    param("Mfull", [POOLP, NG * STOT], f16)
    param("IOTABIG", [P, meta["CMAX"] * P], f16)
    param("IOTACOL", [P, 1], f16)
    param("W1T16", [P, CL1], f16)
    param("W116", [CL1, P], f16)
    param("A1blk", [CL1, 8], f16)
    param("W2T16", [CL1, CL2], f16)
    param("W216a", [P, CL1], f16)
    param("W216b", [CL2 - P, CL1], f16)
    param("A2blka", [P, 8], f16)
    param("A2blkb", [CL2 - P, 8], f16)
    param("b1row", [P, CL1], f32)
    param("b2row", [P, OUT], f32)
    param("blrow", [NG, NCLS], f32)
    param("WlT32", [OUT, NCLS], f32)
    out_ext = param("out_logits", [NG, NCLS], f32, out=True)

    rg = [list(range(NC))]

    with tile.TileContext(nc) as tc:
        with tc.tile_pool(name="const", bufs=1) as cp, \
             tc.tile_pool(name="dram", bufs=1, space="DRAM") as dp:

            # ---- constants to SBUF
            def ld(name, shape, dt):
                t_ = cp.tile(list(shape), dt, name="c_" + name)
                nc.sync.dma_start(t_[:], pr[name][:])
                return t_
            xT = ld("xT16", [P, TPAD], f16)
            iotaB = ld("IOTABIG", [P, meta["CMAX"] * P], f16)
            W1T = ld("W1T16", [P, CL1], f16)
            W116 = ld("W116", [CL1, P], f16)
            A1blk = ld("A1blk", [CL1, 8], f16)
            W2T = ld("W2T16", [CL1, CL2], f16)
            W216a = ld("W216a", [P, CL1], f16)
            W216b = ld("W216b", [CL2 - P, CL1], f16)
            A2blka = ld("A2blka", [P, 8], f16)
            A2blkb = ld("A2blkb", [CL2 - P, 8], f16)
            b1row = ld("b1row", [P, CL1], f32)
            b2row = ld("b2row", [P, OUT], f32)
            blrow = ld("blrow", [NG, NCLS], f32)
            WlT32 = ld("WlT32", [OUT, NCLS], f32)
            Mfull = ld("Mfull", [POOLP, NG * STOT], f16)

            ident = cp.tile([P, P], f32)
            make_identity(nc, ident[:])
            iotaC = ld("IOTACOL", [P, 1], f16)
            onesW = cp.tile([P, meta["CMAX"] * P], f16)
            nc.vector.memset(onesW[:], 1.0)

            # ---- B1/B2 fold matrices (al = x @ B1 ; al2 = u @ B2)
            B1 = cp.tile([P, 8], f16)
            B2 = cp.tile([P, 8], f16)
            with tc.tile_pool(name="cps", bufs=1, space="PSUM") as cps:
                B1ps = cps.tile([P, 8], f32)
                nc.tensor.matmul(B1ps[:], lhsT=W116[:], rhs=A1blk[:], start=True, stop=True)
                nc.vector.tensor_copy(out=B1[:], in_=B1ps[:])
                B2ps = cps.tile([P, 8], f32)
                nc.tensor.matmul(B2ps[:], lhsT=W216a[:], rhs=A2blka[:], start=True, stop=False)
                nc.tensor.matmul(B2ps[:], lhsT=W216b[:], rhs=A2blkb[:], start=False, stop=True)
                nc.vector.tensor_copy(out=B2[:], in_=B2ps[:])

            # ---- DRAM internals (padded rows)
            tb1_loc = dp.tile([TPAD, R1], f16)
            tb1_full = dp.tile([VPAD, R1], f16)
            tb2_loc = dp.tile([TPAD, R2], f16)
            tb2_full = dp.tile([VPAD, R2], f16)
            ad1P = dp.tile([TPAD, P], f16)      # compact 256B rows: [as|ad|junk]
            ad2P = dp.tile([TPAD, P], f16)
            ar_in = dp.tile([POOLP, NG], f32)
            ar_out = dp.tile([POOLP, NG], f32)

            # ================= phase B: table1 build =================
            with tc.tile_pool(name="phB", bufs=3) as pb, \
                 tc.tile_pool(name="phBps", bufs=2, space="PSUM") as pbps:
                for t in range(TILES):
                    xsl = xT[:, t * P:(t + 1) * P]
                    h1ps = pbps.tile([P, CL1], f32, tag="h1ps")
                    nc.tensor.matmul(h1ps[:], lhsT=xsl, rhs=W1T[:], start=True, stop=True)
                    alps = pbps.tile([P, 8], f32, tag="alps")
                    nc.tensor.matmul(alps[:], lhsT=xsl, rhs=B1[:], start=True, stop=True)
                    h116 = pb.tile([P, CL1], f16, tag="h116")
                    nc.vector.tensor_copy(out=h116[:], in_=h1ps[:])
                    al16 = pb.tile([P, 8], f16, tag="al16")
                    nc.vector.tensor_copy(out=al16[:], in_=alps[:])
                    nc.sync.dma_start(tb1_loc[t * P:(t + 1) * P, 0:CL1], h116[:])
                    nc.sync.dma_start(tb1_loc[t * P:(t + 1) * P, CL1:CL1 + 8], al16[:])
                    nc.sync.dma_start(ad1P[t * P:(t + 1) * P, 0:8], al16[:])

            for c in range(NCHUNK):
                nc.gpsimd.collective_compute(
                    "AllGather", mybir.AluOpType.bypass, replica_groups=rg,
                    ins=[tb1_loc[c * CH:(c + 1) * CH, :].opt()],
                    outs=[tb1_full[c * NC * CH:(c + 1) * NC * CH, :].opt()])

            # ================= edge phases =================
            def edge_tile(pe, pp, ppad, t, CL, ROWW, tfull, tfull_loc, tloc, epilogue):
                RW = CL + 4
                WA, WB, C = WAl[t], WBl[t], Cl[t]
                io, do = idx_off[t], dl_off[t]
                it = pe.tile([P, C * 8], i16, tag="it")
                nc.sync.dma_start(it[:], pr["gidx"][:, io:io + C * 8])
                dlt = pe.tile([P, C], f16, tag="dlt")
                nc.sync.dma_start(dlt[:], pr["dlocf"][:, do:do + C])
                dlR = pe.tile([P, C * P], f16, tag="dlR")
                nc.sync.dma_start(
                    dlR[:], pr["dlocflat"][:, do * P:(do + C) * P])
                adt = pe.tile([P, 8], f16, tag="adt")
                nc.sync.dma_start(adt[:], tloc[t * P:(t + 1) * P, 0:8])
                hloc = pe.tile([P, CL], f16, tag="hloc")
                nc.sync.dma_start(hloc[:], tfull_loc[t * P:(t + 1) * P, 0:CL])

                hg = pe.tile([P, C * ROWW], f16, tag="hg")
                nc.gpsimd.dma_gather(
                    out_ap=hg[:, 0:WA * ROWW].rearrange("p (w r) -> p w r", w=WA),
                    in_ap=tfull[0:HALF, :], idxs_ap=it[:, 0:WA * 8],
                    num_idxs=WA * P, num_idxs_reg=WA * P, elem_size=ROWW,
                    single_packet=False)
                nc.gpsimd.dma_gather(
                    out_ap=hg[:, WA * ROWW:].rearrange("p (w r) -> p w r", w=WB),
                    in_ap=tfull[HALF:VPAD, :], idxs_ap=it[:, WA * 8:C * 8],
                    num_idxs=WB * P, num_idxs_reg=WB * P, elem_size=ROWW,
                    single_packet=False)

                hg3 = hg[:].rearrange("p (w r) -> p w r", w=C)

                # slot-major mask: expand dloc on Scalar engine, compare on DVE
                dlE = pe.tile([P, C * P], f16, tag="dlE")
                nc.scalar.activation(
                    dlE[:].rearrange("p (c d) -> p c d", c=C),
                    dlt[:, :, None].to_broadcast([P, C, P]), Act.Copy)
                mask = pe.tile([P, C * P], f16, tag="mask")
                nc.vector.tensor_tensor(out=mask[:], in0=dlE[:],
                                        in1=iotaB[:, 0:C * P], op=Alu.is_equal)

                # transposed mask (partitions = dst) for the ad lookup
                maskT = pe.tile([P, C * P], f16, tag="maskT")
                nc.vector.scalar_tensor_tensor(
                    out=maskT[:], in0=dlR[:], scalar=iotaC[:, 0:1],
                    in1=onesW[:, 0:C * P], op0=Alu.is_equal, op1=Alu.mult)
                adps = ppad.tile([P, C * 4], f32, tag="adps")
                for j in range(C):
                    nc.tensor.matmul(adps[:, j * 4:(j + 1) * 4],
                                     lhsT=maskT[:, j * P:(j + 1) * P],
                                     rhs=adt[:, 4:8], start=True, stop=True)

                # attention logits -> ex
                sc = pe.tile([P, C * 4], f32, tag="sc")
                nc.vector.tensor_tensor(
                    out=sc[:].rearrange("p (w h) -> p w h", w=C),
                    in0=hg3[:, :, CL:CL + 4], in1=adps[:].rearrange("p (w h) -> p w h", w=C),
                    op=Alu.add)
                lr = pe.tile([P, C * 4], f32, tag="lr")
                nc.vector.scalar_tensor_tensor(out=lr[:], in0=sc[:], scalar=NEG,
                                               in1=sc[:], op0=Alu.mult, op1=Alu.max)
                ex = pe.tile([P, C * 4], f16, tag="ex")
                nc.scalar.activation(ex[:], lr[:], Act.Exp)
                ex3 = ex[:].rearrange("p (w h) -> p w h", w=C)

                # rhs: head-interleaved h * ex (2x eligible), ex columns for
                # the denominators copied in on the Scalar engine
                rhs = pe.tile([P, C * RW], f16, tag="rhs")
                rhs3 = rhs[:].rearrange("p (w r) -> p w r", w=C)
                nc.vector.tensor_tensor(
                    out=rhs3[:, :, 0:CL].rearrange("p w (q h) -> p w q h", h=HEADS),
                    in0=hg3[:, :, 0:CL].rearrange("p w (q h) -> p w q h", h=HEADS),
                    in1=ex3[:, :, None, :].to_broadcast([P, C, CL // HEADS, HEADS]),
                    op=Alu.mult)
                nc.scalar.activation(rhs3[:, :, CL:CL + 4], ex3, Act.Copy)

                # self-loop contribution from local rows (no gather)
                scS = pe.tile([P, 4], f32, tag="scS")
                nc.vector.tensor_tensor(out=scS[:], in0=adt[:, 0:4],
                                        in1=adt[:, 4:8], op=Alu.add)
                lrS = pe.tile([P, 4], f32, tag="lrS")
                nc.vector.scalar_tensor_tensor(out=lrS[:], in0=scS[:], scalar=NEG,
                                               in1=scS[:], op0=Alu.mult, op1=Alu.max)
                exS = pe.tile([P, 4], f16, tag="exS")
                nc.scalar.activation(exS[:], lrS[:], Act.Exp)
                rhsS = pe.tile([P, RW], f32, tag="rhsS")
                nc.vector.tensor_tensor(
                    out=rhsS[:, 0:CL].rearrange("p (q h) -> p q h", h=HEADS),
                    in0=hloc[:].rearrange("p (q h) -> p q h", h=HEADS),
                    in1=exS[:, None, :].to_broadcast([P, CL // HEADS, HEADS]),
                    op=Alu.mult)
                nc.scalar.activation(rhsS[:, CL:CL + 4], exS[:], Act.Copy)

                num = pp.tile([P, RW], f32, tag="num")
                for j in range(C):
                    nc.tensor.matmul(num[:], lhsT=mask[:, j * P:(j + 1) * P],
                                     rhs=rhs[:, j * RW:(j + 1) * RW],
                                     start=(j == 0), stop=(j == C - 1))
                nc.vector.tensor_tensor(out=num[:], in0=num[:], in1=rhsS[:],
                                        op=Alu.add)
                epilogue(t, num)

            # ---- layer 1 (+ table2 build in epilogue)
            with tc.tile_pool(name="ph1", bufs=2) as p1, \
                 tc.tile_pool(name="ph1b", bufs=2) as p1b, \
                 tc.tile_pool(name="ph1num", bufs=2, space="PSUM") as p1num, \
                 tc.tile_pool(name="ph1ad", bufs=2, space="PSUM") as p1ad, \
                 tc.tile_pool(name="ph1ps", bufs=1, space="PSUM") as p1ps:

                def epi1(t, num):
                    den = p1b.tile([P, 4], f32, tag="den")
                    nc.vector.tensor_scalar_add(out=den[:], in0=num[:, CL1:CL1 + 4], scalar1=1e-16)
                    rden = p1b.tile([P, 4], f32, tag="rden")
                    nc.vector.reciprocal(rden[:], den[:])
                    rdE = p1b.tile([P, CL1], f32, tag="rdE")
                    nc.scalar.activation(
                        rdE[:].rearrange("p (q h) -> p q h", h=HEADS),
                        rden[:, None, :].to_broadcast([P, CL1 // HEADS, HEADS]),
                        Act.Copy)
                    mu = p1b.tile([P, CL1], f32, tag="mu")
                    nc.vector.tensor_tensor(out=mu[:], in0=num[:, 0:CL1], in1=rdE[:],
                                            op=Alu.mult)
                    mb = p1b.tile([P, CL1], f32, tag="mb")
                    nc.vector.tensor_tensor(out=mb[:], in0=mu[:], in1=b1row[:], op=Alu.add)
                    ur = p1b.tile([P, CL1], f32, tag="ur")
                    nc.scalar.activation(ur[:], mb[:], Act.Relu)
                    uTps = p1ps.tile([P, P], f32, tag="uTps")
                    nc.tensor.transpose(out=uTps[:], in_=ur[:], identity=ident[:])
                    uT16 = p1b.tile([P, P], f16, tag="uT16")
                    nc.scalar.activation(uT16[:], uTps[:], Act.Copy)
                    h2ps = p1ps.tile([P, CL2], f32, tag="h2ps")
                    nc.tensor.matmul(h2ps[:], lhsT=uT16[:], rhs=W2T[:], start=True, stop=True)
                    al2ps = p1ps.tile([P, 8], f32, tag="al2ps")
                    nc.tensor.matmul(al2ps[:], lhsT=uT16[:], rhs=B2[:], start=True, stop=True)
                    h216 = p1b.tile([P, CL2], f16, tag="h216")
                    nc.scalar.activation(h216[:], h2ps[:], Act.Copy)
                    al216 = p1b.tile([P, 8], f16, tag="al216")
                    nc.vector.tensor_copy(out=al216[:], in_=al2ps[:])
                    nc.sync.dma_start(tb2_loc[t * P:(t + 1) * P, 0:CL2], h216[:])
                    nc.sync.dma_start(tb2_loc[t * P:(t + 1) * P, CL2:CL2 + 8], al216[:])
                    nc.sync.dma_start(ad2P[t * P:(t + 1) * P, 0:8], al216[:])

                for t in range(TILES):
                    edge_tile(p1, p1num, p1ad, t, CL1, R1, tb1_full, tb1_loc, ad1P, epi1)

            for c in range(NCHUNK):
                nc.gpsimd.collective_compute(
                    "AllGather", mybir.AluOpType.bypass, replica_groups=rg,
                    ins=[tb2_loc[c * CH:(c + 1) * CH, :].opt()],
                    outs=[tb2_full[c * NC * CH:(c + 1) * NC * CH, :].opt()])

            # ---- layer 2 + pooling
            partial = cp.tile([POOLP, STOT], f32)
            with tc.tile_pool(name="ph2", bufs=2) as p2, \
                 tc.tile_pool(name="ph2b", bufs=2) as p2b, \
                 tc.tile_pool(name="ph2num", bufs=2, space="PSUM") as p2num, \
                 tc.tile_pool(name="ph2ad", bufs=2, space="PSUM") as p2ad, \
                 tc.tile_pool(name="ph2ps", bufs=1, space="PSUM") as p2ps:

                def epi2(t, num):
                    den = p2b.tile([P, 4], f32, tag="den2")
                    nc.vector.tensor_scalar_add(out=den[:], in0=num[:, CL2:CL2 + 4], scalar1=1e-16)
                    rden = p2b.tile([P, 4], f32, tag="rden2")
                    nc.vector.reciprocal(rden[:], den[:])
                    rdE = p2b.tile([P, CL2], f32, tag="rdE2")
                    nc.scalar.activation(
                        rdE[:].rearrange("p (q h) -> p q h", h=HEADS),
                        rden[:, None, :].to_broadcast([P, CL2 // HEADS, HEADS]),
                        Act.Copy)
                    mo = p2b.tile([P, CL2], f32, tag="mo")
                    nc.vector.tensor_tensor(out=mo[:], in0=num[:, 0:CL2], in1=rdE[:],
                                            op=Alu.mult)
                    hsum = p2b.tile([P, OUT], f32, tag="hsum")
                    nc.vector.reduce_sum(hsum[:],
                                         mo[:].rearrange("p (q h) -> p q h", h=HEADS),
                                         axis=mybir.AxisListType.X)
                    o2 = p2b.tile([P, OUT], f32, tag="o2")
                    nc.vector.scalar_tensor_tensor(
                        out=o2[:], in0=hsum[:], scalar=1.0 / HEADS,
                        in1=b2row[:], op0=Alu.mult, op1=Alu.add)
                    o2Tps = p2ps.tile([OUT, P], f32, tag="o2Tps")
                    nc.tensor.transpose(out=o2Tps[:], in_=o2[:], identity=ident[:])
                    o2T = p2b.tile([OUT, P], f32, tag="o2T")
                    nc.scalar.activation(o2T[:], o2Tps[:], Act.Copy)
                    sm = p2b.tile([POOLP, SEG * P], f16, tag="sm")
                    nc.sync.dma_start(sm[:], pr["segmask"][t * POOLP:(t + 1) * POOLP, :])
                    for s in range(SEG):
                        tmp = p2b.tile([POOLP, P], f32, tag="ptmp")
                        nc.vector.scalar_tensor_tensor(
                            out=tmp[:], in0=o2T[:], scalar=OFF,
                            in1=sm[:, s * P:(s + 1) * P], op0=Alu.add, op1=Alu.mult)
                        nc.vector.reduce_max(partial[:, t * SEG + s:t * SEG + s + 1],
                                             tmp[:], axis=mybir.AxisListType.X)

                for t in range(TILES):
                    edge_tile(p2, p2num, p2ad, t, CL2, R2, tb2_full, tb2_loc, ad2P, epi2)

            # ---- combine partials -> per-graph max -> AllReduce -> head
            with tc.tile_pool(name="fin", bufs=1) as fp, \
                 tc.tile_pool(name="finps", bufs=1, space="PSUM") as fps:
                tmpg = fp.tile([POOLP, NG * STOT], f32)
                nc.vector.scalar_tensor_tensor(
                    out=tmpg[:], in0=partial[:][:, None, :].to_broadcast([POOLP, NG, STOT]),
                    scalar=0.0, in1=Mfull[:].rearrange("p (g s) -> p g s", g=NG),
                    op0=Alu.add, op1=Alu.mult)
                pooled = fp.tile([POOLP, NG], f32)
                nc.vector.reduce_max(pooled[:],
                                     tmpg[:].rearrange("p (g s) -> p g s", g=NG),
                                     axis=mybir.AxisListType.X)
                pooled2 = fp.tile([POOLP, NG], f32)
                nc.vector.tensor_scalar_add(out=pooled2[:], in0=pooled[:], scalar1=-OFF)
                nc.sync.dma_start(ar_in[:], pooled2[:])
                nc.gpsimd.collective_compute(
                    "AllReduce", mybir.AluOpType.max, replica_groups=rg,
                    ins=[ar_in[:].opt()], outs=[ar_out[:].opt()])
                pooledF = fp.tile([POOLP, NG], f32)
                nc.sync.dma_start(pooledF[:], ar_out[:])
                lps = fps.tile([NG, NCLS], f32)
                nc.tensor.matmul(lps[:], lhsT=pooledF[:], rhs=WlT32[:], start=True, stop=True)
                logits = fp.tile([NG, NCLS], f32)
                nc.vector.tensor_tensor(out=logits[:], in0=lps[:], in1=blrow[:], op=Alu.add)
                nc.sync.dma_start(out_ext[:], logits[:])

    nc.compile()
    return nc


# ---------------------------------------------------------------- runner
def _make_in_maps(cfg, per_core, shared):
    maps = []
    for k in range(cfg["NC"]):
        m = dict(shared)
        m.update(per_core[k])
        maps.append({k_: np.ascontiguousarray(v) for k_, v in m.items()})
    return maps


def _kernel_impl(inputs, trace=False, trace_kwargs=None):
    cfg = _derive(FULL_CFG)
    x = np.asarray(inputs["x"], np.float32)
    edge_index = np.asarray(inputs["edge_index"]).astype(np.int64)
    batch = np.asarray(inputs["batch"]).astype(np.int64)
    meta, per_core = _preprocess(cfg, x, edge_index, batch)
    shared = _weights_host(
        cfg, meta,
        np.asarray(inputs["W1"], np.float32), np.asarray(inputs["a_src1"], np.float32),
        np.asarray(inputs["a_dst1"], np.float32), np.asarray(inputs["b1"], np.float32),
        np.asarray(inputs["W2"], np.float32), np.asarray(inputs["a_src2"], np.float32),
        np.asarray(inputs["a_dst2"], np.float32), np.asarray(inputs["b2"], np.float32),
        np.asarray(inputs["Wl"], np.float32), np.asarray(inputs["bl"], np.float32))
    nc = _build(cfg, meta)
    in_maps = _make_in_maps(cfg, per_core, shared)
    from concourse.bass_utils import run_bass_kernel_spmd
    res = run_bass_kernel_spmd(nc, in_maps, core_ids=list(range(cfg["NC"])),
                               trace=trace, **(trace_kwargs or {}))
    return np.asarray(res.results[0]["out_logits"], np.float32), res


def kernel(**inputs):
    return _kernel_impl(inputs)[0]
